# revision 17
# baseline (speedup 1.0000x reference)
"""Trainium2 Bass kernel for nn_DecoderPreLN2 (4-layer cross-attention decoder).

Sharding: data-parallel over batch N=8 across 8 NeuronCores (1 element/core).
Per-core dataflow is fully "transposed" (D-major): activations live as x^T
[D=1024 partitions(8 tiles), T free] so every matmul keeps weights stationary
and no on-device transposes are needed.

v2 design (vs fp32r baseline):
  - All matmuls bf16 (measured same row-rate as fp32r but lighter LDWEIGHTS
    and half the DMA); PSUM accumulates f32.
  - The PE is kept *continuously busy* so the HAM clock gate stays at 8/8
    (2.4 GHz): K/V projections of layer i+1 are interleaved as "fill" matmuls
    into the ACT-gated gaps of attention(i).
  - exp(mask) is folded into the V ones-column (host computes exp(mask);
    the V-copy scales rows by it), so the score exp needs no bias operand
    and two k-tiles are merged per ACTIVATE (bf16 out).
  - LN affine + 1/sqrt(HD) folded into wq (host). LN statistics are computed
    on raw attention output via PE ones-matmuls into a proj-pool psum tile;
    the per-token LN scalars are applied as a *post*-fixup on the Q
    projection output (q = R*(qraw + c) + M*s + bq), so Q matmuls never wait
    on the LN chain.
  - V-bias bv is folded into the next layer's LN stats (softmax weights sum
    to 1); the last layer adds bv explicitly.
  - rstd via DVE-only fast-inverse-sqrt (bit trick + 2 Newton steps): no
    ACT table switching (ACT only ever loads the exp set).
  - Softmax denominator via the (scaled) ones-column of V; normalize =
    DVE reciprocal + GPSIMD partition_broadcast + DVE multiply.
  - Last layer streams the f32 output to HBM per head-pair tile.
"""

import os
import sys

sys.path.insert(0, "/opt/trn_rl_repo")

DEBUG = bool(int(os.environ.get("KERNEL_DEBUG", "0")))

import numpy as np
import ml_dtypes

import concourse.bass as bass
import concourse.tile as tile
from concourse import bacc, mybir
from concourse.bass import ts
from concourse.bass_utils import run_bass_kernel_spmd

L, D, H, HD = 4, 1024, 16, 64
TQ, TK, NB = 512, 1024, 8
DT = D // 128  # 8 d-tiles
KT = TK // 128  # 8 k-token tiles
EPS = 1e-5

F32 = mybir.dt.float32
BF16 = mybir.dt.bfloat16
I32 = mybir.dt.int32
AF = mybir.ActivationFunctionType
OP = mybir.AluOpType

RSQRT_MAGIC = 0x5F3759DF

_PROGRAM = {}


def build_program(spec=(True, True, True)):
    """spec = (skip_bq, skip_bvp, cval_ones) input-derived specializations."""
    global _PROGRAM
    if spec in _PROGRAM:
        return _PROGRAM[spec]
    SKIP_BQ, SKIP_BVP, CVAL_ONES = spec

    nc = bacc.Bacc(
        "TRN2", target_bir_lowering=False, debug=False,
        dynamic_dma_scratch_size=2048,
    )

    xt0 = nc.dram_tensor("xt0", [D, TQ], BF16, kind="ExternalInput").ap()
    enct = nc.dram_tensor("enct", [D, TK], BF16, kind="ExternalInput").ap()
    cvald = nc.dram_tensor("cvald", [128, KT], F32, kind="ExternalInput").ap()
    wqd = nc.dram_tensor("wqd", [L, D, D], BF16, kind="ExternalInput").ap()
    wkd = nc.dram_tensor("wkd", [L, D, D], BF16, kind="ExternalInput").ap()
    wvd = nc.dram_tensor("wvd", [L, D, D], BF16, kind="ExternalInput").ap()
    # per-layer f32 columns: 0-7 bk, 8-15 cq, 16-23 sq, 24-31 bq2,
    # [0,32]=sum(bvp), [0,33]=D*EPS, 34-41 bvp (pcol)
    bcons = nc.dram_tensor("bcons", [L, 128, 42], F32, kind="ExternalInput").ap()
    bvfd = nc.dram_tensor("bvfd", [128, DT], F32, kind="ExternalInput").ap()
    outd = nc.dram_tensor("outd", [D, TQ], F32, kind="ExternalOutput").ap()
    outr = outd.rearrange("(j p) t -> p j t", p=128)
    if DEBUG:
        dbg_kt = nc.dram_tensor("dbg_kt", [128, DT, TK], BF16, kind="ExternalOutput").ap()
        dbg_v = nc.dram_tensor("dbg_v", [128, KT, H, HD + 1], BF16, kind="ExternalOutput").ap()
        dbg_qt = nc.dram_tensor("dbg_qt", [128, DT, TQ], BF16, kind="ExternalOutput").ap()
        dbg_R = nc.dram_tensor("dbg_R", [2, TQ], F32, kind="ExternalOutput").ap()
        dbg_x1 = nc.dram_tensor("dbg_x1", [128, DT, TQ], BF16, kind="ExternalOutput").ap()
        dbg_pt = nc.dram_tensor("dbg_pt", [128, 2, 2, TQ], BF16, kind="ExternalOutput").ap()

    from contextlib import ExitStack

    with tile.TileContext(nc) as tc, ExitStack() as es:
            persist = es.enter_context(tc.tile_pool(name="persist", bufs=1))
            xp = es.enter_context(tc.tile_pool(name="xp", bufs=2))
            ktp = es.enter_context(tc.tile_pool(name="ktp", bufs=2))
            vp = es.enter_context(tc.tile_pool(name="vp", bufs=2))
            qtp = es.enter_context(tc.tile_pool(name="qtp", bufs=1))
            wkp = es.enter_context(tc.tile_pool(name="wkp", bufs=1))
            wvp = es.enter_context(tc.tile_pool(name="wvp", bufs=1))
            wqp = es.enter_context(tc.tile_pool(name="wqp", bufs=1))
            ptp = es.enter_context(tc.tile_pool(name="ptp", bufs=4))
            sqp = es.enter_context(tc.tile_pool(name="sqp", bufs=8))
            tbp = es.enter_context(tc.tile_pool(name="tbp", bufs=2))
            biasp = es.enter_context(tc.tile_pool(name="biasp", bufs=2))
            smalls = es.enter_context(tc.tile_pool(name="smalls", bufs=1))
            rmp = es.enter_context(tc.tile_pool(name="rmp", bufs=1))
            tqp = es.enter_context(tc.tile_pool(name="tqp", bufs=1))
            recipp = es.enter_context(tc.tile_pool(name="recipp", bufs=2))
            bcp = es.enter_context(tc.tile_pool(name="bcp", bufs=2))
            stgp = es.enter_context(tc.tile_pool(name="stgp", bufs=2))
            proj_ps = es.enter_context(
                tc.tile_pool(name="proj_ps", bufs=2, space="PSUM"))
            sc_ps = es.enter_context(
                tc.tile_pool(name="sc_ps", bufs=2, space="PSUM"))
            avA_ps = es.enter_context(
                tc.tile_pool(name="avA_ps", bufs=1, space="PSUM"))
            avB_ps = es.enter_context(
                tc.tile_pool(name="avB_ps", bufs=1, space="PSUM"))
            # ---- persistent tiles; startup DMAs chunked so the first
            # K-projection matmuls can begin after ~2 d-tiles arrive ----
            enc_sb = persist.tile([128, DT, TK], BF16, tag="enc")
            encr = enct.rearrange("(j p) t -> p j t", p=128)
            wk_sb = wkp.tile([128, DT, D], BF16, tag="wk")
            wkr0 = wkd[0].rearrange("(j p) o -> p j o", p=128)
            for k in range(DT):
                nc.sync.dma_start(out=enc_sb[:, k, :], in_=encr[:, k, :])
                nc.sync.dma_start(out=wk_sb[:, k, :], in_=wkr0[:, k, :])
            cval_sb = persist.tile([128, KT], F32, tag="cval")
            nc.sync.dma_start(out=cval_sb[:], in_=cvald[:])
            ones1 = persist.tile([128, 1], BF16, tag="ones1")
            nc.vector.memset(ones1[:], 1.0)
            bvf_sb = persist.tile([128, DT], F32, tag="bvf")
            nc.sync.dma_start(out=bvf_sb[:], in_=bvfd[:])

            xb = [xp.tile([128, DT, TQ], BF16, tag="x", name=f"x{j}")
                  for j in range(2)]
            nc.sync.dma_start(
                out=xb[0][:], in_=xt0.rearrange("(j p) t -> p j t", p=128)
            )
            ktb = [ktp.tile([128, DT, TK], BF16, tag="kt", name=f"kt{j}")
                   for j in range(2)]
            vb = [vp.tile([128, KT, H, HD + 1], BF16, tag="vaug", name=f"v{j}")
                  for j in range(2)]
            qt_sb = qtp.tile([128, DT, TQ], BF16, tag="qt")
            qraw_sb = qtp.tile([128, DT, TQ], BF16, tag="qraw")
            # scaled-ones column of v_aug (den weights): exp(mask) per token
            for j in range(2):
                for h in range(H):
                    nc.vector.tensor_copy(vb[j][:, :, h, HD : HD + 1], cval_sb[:])

            wv_sb = wvp.tile([128, DT, D], BF16, tag="wv")
            wq_sb = wqp.tile([128, DT, D], BF16, tag="wq")

            def dma_w(dst, wd, i):
                nc.sync.dma_start(
                    out=dst[:], in_=wd[i].rearrange("(j p) o -> p j o", p=128)
                )

            bc_sb = [None, None]

            def dma_consts(i):
                b = biasp.tile([128, 42], F32, tag="bcons")
                nc.sync.dma_start(out=b[:], in_=bcons[i])
                bc_sb[i % 2] = b

            # ---- fill generator: K+V projections for layer i ----
            def kv_gen(i):
                kt_dst = ktb[i % 2]
                v_dst = vb[i % 2]
                bcs = bc_sb[i % 2]
                for half in range(2):
                    for nl in range(4):
                        n = half * 4 + nl
                        for c in range(2):
                            ps = proj_ps.tile([128, 512], F32, tag="proj")
                            for k in range(DT):
                                nc.tensor.matmul(
                                    ps[:],
                                    wk_sb[:, k, ts(half, 512)][:, ts(nl, 128)],
                                    enc_sb[:, k, ts(c, 512)],
                                    start=(k == 0), stop=(k == DT - 1),
                                )
                                yield 1
                            nc.scalar.add(
                                kt_dst[:, n, ts(c, 512)], ps[:],
                                bcs[:, n : n + 1],
                            )
                    for m in range(KT):
                        ps = proj_ps.tile([128, 512], F32, tag="proj")
                        for k in range(DT):
                            nc.tensor.matmul(
                                ps[:], enc_sb[:, k, ts(m, 128)],
                                wv_sb[:, k, ts(half, 512)],
                                start=(k == 0), stop=(k == DT - 1),
                            )
                            yield 1
                        # scale token rows by exp(mask) while copying
                        if CVAL_ONES:
                            nc.scalar.copy(
                                v_dst[:, m, ts(half, 8), 0:HD],
                                ps[:].rearrange("p (h e) -> p h e", h=8),
                            )
                        else:
                            nc.vector.tensor_scalar_mul(
                                v_dst[:, m, ts(half, 8), 0:HD],
                                ps[:].rearrange("p (h e) -> p h e", h=8),
                                cval_sb[:, m : m + 1],
                            )

            _DONE = object()

            def pull(g, n):
                for _ in range(n):
                    if next(g, _DONE) is _DONE:
                        return False
                return True

            def drain(g):
                while next(g, _DONE) is not _DONE:
                    pass

            # ---- LN stats + smalls + Q projection + fixup for layer i ----
            def ln_q(i, sq_tiles):
                x = xb[i % 2]
                bcs = bc_sb[i % 2]
                lnt = proj_ps.tile([128, 512], F32, tag="proj")
                for k in range(DT):
                    nc.tensor.matmul(
                        lnt[0:1, :], ones1[:], x[:, k, :],
                        start=(k == 0), stop=(k == DT - 1),
                    )
                for k in range(DT):
                    nc.tensor.matmul(
                        lnt[32:33, :], ones1[:], sq_tiles[k][:],
                        start=(k == 0), stop=(k == DT - 1),
                    )
                # smalls: mu, var, rstd (fast inverse sqrt), M = -mu*rstd
                mu = smalls.tile([1, TQ], F32, tag="mu")
                nc.vector.tensor_scalar(
                    mu[:], lnt[0:1, :], bcs[0:1, 32:33], 1.0 / D,
                    op0=OP.add, op1=OP.mult,
                )
                e2 = smalls.tile([1, TQ], F32, tag="e2")
                nc.vector.tensor_scalar(
                    e2[:], lnt[32:33, :], bcs[0:1, 33:34], 1.0 / D,
                    op0=OP.add, op1=OP.mult,
                )
                msq = smalls.tile([1, TQ], F32, tag="msq")
                nc.vector.tensor_mul(msq[:], mu[:], mu[:])
                # e2 <- var = e2 - mu^2   (EPS folded into bcs[33] host-side)
                nc.vector.scalar_tensor_tensor(
                    e2[:], msq[:], -1.0, e2[:], op0=OP.mult, op1=OP.add
                )
                # fast inverse sqrt: y = bitcast(MAGIC - (v_int >> 1))
                y = smalls.tile([1, TQ], F32, tag="y")
                nc.vector.tensor_scalar(
                    y[:].bitcast(I32), e2[:].bitcast(I32), 1, -1,
                    op0=OP.logical_shift_right, op1=OP.bitwise_xor,
                )
                nc.vector.tensor_scalar_add(
                    y[:].bitcast(I32), y[:].bitcast(I32), RSQRT_MAGIC + 1
                )
                hv = smalls.tile([1, TQ], F32, tag="hv")
                nc.vector.tensor_scalar_mul(hv[:], e2[:], -0.5)
                u = smalls.tile([1, TQ], F32, tag="u")
                for _ in range(1):  # one Newton step (seed err 3.4% -> 0.17%)
                    nc.vector.tensor_mul(u[:], y[:], y[:])
                    nc.vector.tensor_mul(u[:], u[:], hv[:])
                    nc.vector.tensor_scalar_add(u[:], u[:], 1.5)
                    nc.vector.tensor_mul(y[:], y[:], u[:])
                m1 = smalls.tile([1, TQ], F32, tag="m1")
                nc.vector.scalar_tensor_tensor(
                    m1[:], mu[:], -1.0, y[:], op0=OP.mult, op1=OP.mult
                )
                R = rmp.tile([128, TQ], F32, tag="R")
                nc.gpsimd.partition_broadcast(R[:], y[:])
                M = rmp.tile([128, TQ], F32, tag="M")
                nc.gpsimd.partition_broadcast(M[:], m1[:])
                if DEBUG and i == 0:
                    nc.sync.dma_start(out=dbg_R[0:1, :], in_=y[:])
                    nc.sync.dma_start(out=dbg_R[1:2, :], in_=m1[:])

                # Q projection on raw x; consumer is a plain psum->sbuf
                # copy so the proj-psum ring never waits on the LN chain.
                for half in range(2):
                    for nl in range(4):
                        n = half * 4 + nl
                        qp = proj_ps.tile([128, 512], F32, tag="proj")
                        for k in range(DT):
                            nc.tensor.matmul(
                                qp[:],
                                wq_sb[:, k, ts(half, 512)][:, ts(nl, 128)],
                                x[:, k, :],
                                start=(k == 0), stop=(k == DT - 1),
                            )
                        nc.vector.tensor_copy(qraw_sb[:, n, :], qp[:])
                # deferred per-token fixup: q = R*(qraw + c) + M*s (+ bq)
                for n in range(DT):
                    tq = tqp.tile([128, TQ], F32, tag="tq")
                    nc.vector.scalar_tensor_tensor(
                        tq[:], qraw_sb[:, n, :], bcs[:, 8 + n : 9 + n], R[:],
                        op0=OP.add, op1=OP.mult,
                    )
                    nc.vector.scalar_tensor_tensor(
                        qt_sb[:, n, :], M[:], bcs[:, 16 + n : 17 + n],
                        tq[:], op0=OP.mult, op1=OP.add,
                    )
                    if not SKIP_BQ:
                        nc.vector.tensor_scalar_add(
                            qt_sb[:, n, :], qt_sb[:, n, :],
                            bcs[:, 24 + n : 25 + n],
                        )

            # ---- attention for layer i, pulling fill work between groups ----
            def attention(i, fill):
                kt_sb = ktb[i % 2]
                vaug = vb[i % 2]
                last = i == L - 1
                x_next = None if last else xb[(i + 1) % 2]
                sq_tiles = []
                for hp in range(DT):
                    hA, hB = 2 * hp, 2 * hp + 1
                    avA = avA_ps.tile([HD + 1, TQ], F32, tag="avA")
                    avB = avB_ps.tile([HD + 1, TQ], F32, tag="avB")
                    pts = []

                    def av_one(kk):
                        pt = pts[kk]
                        nc.tensor.matmul(
                            avA[:], vaug[:, kk, hA, :], pt[:, 0, :],
                            start=(kk == 0), stop=(kk == KT - 1),
                        )
                        nc.tensor.matmul(
                            avB[:], vaug[:, kk, hB, :], pt[:, 1, :],
                            start=(kk == 0), stop=(kk == KT - 1),
                        )

                    for kk in range(KT):
                        sc = sc_ps.tile([128, 2, TQ], F32, tag="sc")
                        nc.tensor.matmul(
                            sc[:, 0, :],
                            kt_sb[0:64, hp, ts(kk, 128)],
                            qt_sb[0:64, hp, :],
                            start=True, stop=True,
                        )
                        nc.tensor.matmul(
                            sc[:, 1, :],
                            kt_sb[64:128, hp, ts(kk, 128)],
                            qt_sb[64:128, hp, :],
                            start=True, stop=True,
                        )
                        pt = ptp.tile([128, 2, TQ], BF16, tag="pt")
                        nc.scalar.activation(pt[:], sc[:], AF.Exp)
                        pts.append(pt)
                        if kk > 0:
                            av_one(kk - 1)
                        pull(fill, 2 if kk % 2 else 3)
                    av_one(KT - 1)

                    # normalize: x[d,t] = av[d,t] / den[t]
                    st = stgp.tile([128, TQ], F32, tag="stage", name="st") if last else None
                    for av, o in ((avA, 0), (avB, 64)):
                        r0 = recipp.tile([1, TQ], F32, tag="r0")
                        nc.vector.tensor_copy(r0[:], av[HD : HD + 1, :])
                        nc.vector.reciprocal_approx_fast(r0[:], r0[:])
                        bc = bcp.tile([64, TQ], F32, tag="bc")
                        nc.gpsimd.partition_broadcast(bc[:], r0[:])
                        if not last:
                            nc.vector.tensor_tensor(
                                x_next[o : o + 64, hp, :], av[0:HD, :], bc[:],
                                op=OP.mult,
                            )
                        else:
                            nc.vector.tensor_tensor(
                                st[o : o + 64, :], av[0:HD, :], bc[:],
                                op=OP.mult,
                            )
                            nc.vector.tensor_scalar_add(
                                st[o : o + 64, :], st[o : o + 64, :],
                                bvf_sb[o : o + 64, hp : hp + 1],
                            )
                    if last:
                        nc.sync.dma_start(out=outr[:, hp, :], in_=st[:])
                    if not last:
                        sqt = sqp.tile([128, TQ], BF16, tag="sq")
                        if SKIP_BVP:
                            nc.vector.tensor_mul(
                                sqt[:], x_next[:, hp, :], x_next[:, hp, :]
                            )
                        else:
                            bcn = bc_sb[(i + 1) % 2]
                            tb = tbp.tile([128, TQ], BF16, tag="tb")
                            nc.vector.tensor_scalar_add(
                                tb[:], x_next[:, hp, :],
                                bcn[:, 34 + hp : 35 + hp],
                            )
                            nc.vector.tensor_mul(sqt[:], tb[:], tb[:])
                        sq_tiles.append(sqt)
                return sq_tiles

            def dbg_dump(dst, src_tile):
                nc.sync.dma_start(out=dst, in_=src_tile[:])

            # ================= schedule =================
            # prologue: layer 0 projections + LN + Q
            dma_consts(0)
            dma_w(wv_sb, wvd, 0)
            dma_w(wq_sb, wqd, 0)
            g0 = kv_gen(0)
            drain(g0)
            sq0 = []
            for k in range(DT):
                sqt = sqp.tile([128, TQ], BF16, tag="sq")
                if SKIP_BVP:
                    nc.vector.tensor_mul(
                        sqt[:], xb[0][:, k, :], xb[0][:, k, :]
                    )
                else:
                    tb = tbp.tile([128, TQ], BF16, tag="tb")
                    nc.vector.tensor_scalar_add(
                        tb[:], xb[0][:, k, :], bc_sb[0][:, 34 + k : 35 + k]
                    )
                    nc.vector.tensor_mul(sqt[:], tb[:], tb[:])
                sq0.append(sqt)
            ln_q(0, sq0)

            if DEBUG:
                dbg_dump(dbg_kt, ktb[0])
                dbg_dump(dbg_v, vb[0])
                dbg_dump(dbg_qt, qt_sb)
            for i in range(L):
                if i < L - 1:
                    dma_consts(i + 1)
                    dma_w(wk_sb, wkd, i + 1)
                    dma_w(wv_sb, wvd, i + 1)
                    dma_w(wq_sb, wqd, i + 1)
                    fill = kv_gen(i + 1)
                else:
                    fill = iter(())
                sq_tiles = attention(i, fill)
                if DEBUG and i == 0:
                    dbg_dump(dbg_x1, xb[1])
                if i < L - 1:
                    drain(fill)
                    ln_q(i + 1, sq_tiles)

    nc.compile()
    _PROGRAM[spec] = nc
    return nc


def _stage_inputs(input_ids, encoder_state, cross_attn_mask, emb,
                  ln_g, ln_b, wq, bq, wk, bk, wv, bv):
    input_ids = np.asarray(input_ids)
    emb = np.asarray(emb, dtype=np.float32)
    encoder_state = np.asarray(encoder_state, dtype=np.float32)
    cross_attn_mask = np.asarray(cross_attn_mask, dtype=np.float32)
    ln_g = np.asarray(ln_g, dtype=np.float32)
    ln_b = np.asarray(ln_b, dtype=np.float32)
    wq = np.asarray(wq, dtype=np.float32)
    bq = np.asarray(bq, dtype=np.float32)
    wk = np.asarray(wk, dtype=np.float32)
    bk = np.asarray(bk, dtype=np.float32)
    wv = np.asarray(wv, dtype=np.float32)
    bv = np.asarray(bv, dtype=np.float32)

    scale = 1.0 / np.sqrt(HD)
    wq2 = ln_g[:, :, None] * wq * scale  # [L, D, D]
    bq2 = (np.einsum("ld,lde->le", ln_b, wq) + bq) * scale  # [L, D]
    bv_prev = np.concatenate([np.zeros((1, D), np.float32), bv[:-1]], axis=0)
    cq = np.einsum("lde,ld->le", wq2, bv_prev)  # [L, D]
    sq_ = wq2.sum(axis=1)  # [L, D]

    def pcol(a):  # [L, D] -> [L, 128, DT]
        return np.ascontiguousarray(a.reshape(-1, DT, 128).transpose(0, 2, 1))

    bcons = np.zeros((L, 128, 42), np.float32)
    bcons[:, :, 0:8] = pcol(bk)
    bcons[:, :, 8:16] = pcol(cq)
    bcons[:, :, 16:24] = pcol(sq_)
    bcons[:, :, 24:32] = pcol(bq2)
    bcons[:, 0, 32] = bv_prev.sum(axis=1)
    bcons[:, 0, 33] = D * EPS
    bcons[:, :, 34:42] = pcol(bv_prev)
    bf = ml_dtypes.bfloat16

    shared = {
        "wqd": wq2.astype(bf),
        "wkd": wk.astype(bf),
        "wvd": wv.astype(bf),
        "bcons": np.ascontiguousarray(bcons),
        "bvfd": np.ascontiguousarray(pcol(bv[L - 1 : L])[0]),
    }

    x0 = emb[input_ids]  # [NB, TQ, D]
    in_maps = []
    for n in range(NB):
        m = dict(shared)
        m["xt0"] = np.ascontiguousarray(x0[n].T).astype(bf)
        m["enct"] = np.ascontiguousarray(encoder_state[n].T).astype(bf)
        mk = cross_attn_mask[n, 0, 0]  # [TK]
        m["cvald"] = np.ascontiguousarray(
            np.exp(mk).reshape(KT, 128).T.astype(np.float32)
        )
        in_maps.append(m)
    return in_maps


def kernel(**inputs) -> np.ndarray:
    ln_b = np.asarray(inputs["ln_b"], np.float32)
    bq = np.asarray(inputs["bq"], np.float32)
    bv = np.asarray(inputs["bv"], np.float32)
    mask = np.asarray(inputs["cross_attn_mask"], np.float32)
    spec = (
        bool(not ln_b.any() and not bq.any()),
        bool(not bv[:-1].any()),
        bool(not mask.any()),
    )
    nc = build_program(spec)
    in_maps = _stage_inputs(**inputs)
    res = run_bass_kernel_spmd(nc, in_maps, list(range(NB)))
    out = np.stack([np.asarray(res.results[n]["outd"]).T for n in range(NB)])
    return np.ascontiguousarray(out, dtype=np.float32)


if __name__ == "__main__":
    build_program()
    print("program built ok")


# revision 18
# speedup vs baseline: 1.0164x; 1.0164x over previous
"""Trainium2 Bass kernel for nn_DecoderPreLN2 (4-layer cross-attention decoder).

Sharding: data-parallel over batch N=8 across 8 NeuronCores (1 element/core).
Per-core dataflow is fully "transposed" (D-major): activations live as x^T
[D=1024 partitions(8 tiles), T free] so every matmul keeps weights stationary
and no on-device transposes are needed.

v2 design (vs fp32r baseline):
  - All matmuls bf16 (measured same row-rate as fp32r but lighter LDWEIGHTS
    and half the DMA); PSUM accumulates f32.
  - The PE is kept *continuously busy* so the HAM clock gate stays at 8/8
    (2.4 GHz): K/V projections of layer i+1 are interleaved as "fill" matmuls
    into the ACT-gated gaps of attention(i).
  - exp(mask) is folded into the V ones-column (host computes exp(mask);
    the V-copy scales rows by it), so the score exp needs no bias operand
    and two k-tiles are merged per ACTIVATE (bf16 out).
  - LN affine + 1/sqrt(HD) folded into wq (host). LN statistics are computed
    on raw attention output via PE ones-matmuls into a proj-pool psum tile;
    the per-token LN scalars are applied as a *post*-fixup on the Q
    projection output (q = R*(qraw + c) + M*s + bq), so Q matmuls never wait
    on the LN chain.
  - V-bias bv is folded into the next layer's LN stats (softmax weights sum
    to 1); the last layer adds bv explicitly.
  - rstd via DVE-only fast-inverse-sqrt (bit trick + 2 Newton steps): no
    ACT table switching (ACT only ever loads the exp set).
  - Softmax denominator via the (scaled) ones-column of V; normalize =
    DVE reciprocal + GPSIMD partition_broadcast + DVE multiply.
  - Last layer streams the f32 output to HBM per head-pair tile.
"""

import os
import sys

sys.path.insert(0, "/opt/trn_rl_repo")

DEBUG = bool(int(os.environ.get("KERNEL_DEBUG", "0")))

import numpy as np
import ml_dtypes

import concourse.bass as bass
import concourse.tile as tile
from concourse import bacc, mybir
from concourse.bass import ts
from concourse.bass_utils import run_bass_kernel_spmd

L, D, H, HD = 4, 1024, 16, 64
TQ, TK, NB = 512, 1024, 8
DT = D // 128  # 8 d-tiles
KT = TK // 128  # 8 k-token tiles
EPS = 1e-5

F32 = mybir.dt.float32
BF16 = mybir.dt.bfloat16
I32 = mybir.dt.int32
AF = mybir.ActivationFunctionType
OP = mybir.AluOpType

RSQRT_MAGIC = 0x5F3759DF

_PROGRAM = {}


def build_program(spec=(True, True, True)):
    """spec = (skip_bq, skip_bvp, cval_ones) input-derived specializations."""
    global _PROGRAM
    if spec in _PROGRAM:
        return _PROGRAM[spec]
    SKIP_BQ, SKIP_BVP, CVAL_ONES = spec

    nc = bacc.Bacc(
        "TRN2", target_bir_lowering=False, debug=False,
        dynamic_dma_scratch_size=2048,
    )

    xt0 = nc.dram_tensor("xt0", [D, TQ], BF16, kind="ExternalInput").ap()
    enct = nc.dram_tensor("enct", [D, TK], BF16, kind="ExternalInput").ap()
    cvald = nc.dram_tensor("cvald", [128, KT], F32, kind="ExternalInput").ap()
    wqd = nc.dram_tensor("wqd", [L, D, D], BF16, kind="ExternalInput").ap()
    wkd = nc.dram_tensor("wkd", [L, D, D], BF16, kind="ExternalInput").ap()
    wvd = nc.dram_tensor("wvd", [L, D, D], BF16, kind="ExternalInput").ap()
    # per-layer f32 columns: 0-7 bk, 8-15 cq, 16-23 sq, 24-31 bq2,
    # [0,32]=sum(bvp), [0,33]=D*EPS, 34-41 bvp (pcol)
    bcons = nc.dram_tensor("bcons", [L, 128, 42], F32, kind="ExternalInput").ap()
    bvfd = nc.dram_tensor("bvfd", [128, DT], F32, kind="ExternalInput").ap()
    outd = nc.dram_tensor("outd", [D, TQ], F32, kind="ExternalOutput").ap()
    outr = outd.rearrange("(j p) t -> p j t", p=128)
    if DEBUG:
        dbg_kt = nc.dram_tensor("dbg_kt", [128, DT, TK], BF16, kind="ExternalOutput").ap()
        dbg_v = nc.dram_tensor("dbg_v", [128, KT, H, HD + 1], BF16, kind="ExternalOutput").ap()
        dbg_qt = nc.dram_tensor("dbg_qt", [128, DT, TQ], BF16, kind="ExternalOutput").ap()
        dbg_R = nc.dram_tensor("dbg_R", [2, TQ], F32, kind="ExternalOutput").ap()
        dbg_x1 = nc.dram_tensor("dbg_x1", [128, DT, TQ], BF16, kind="ExternalOutput").ap()
        dbg_pt = nc.dram_tensor("dbg_pt", [128, 2, 2, TQ], BF16, kind="ExternalOutput").ap()

    from contextlib import ExitStack

    with tile.TileContext(nc) as tc, ExitStack() as es:
            persist = es.enter_context(tc.tile_pool(name="persist", bufs=1))
            xp = es.enter_context(tc.tile_pool(name="xp", bufs=2))
            ktp = es.enter_context(tc.tile_pool(name="ktp", bufs=2))
            vp = es.enter_context(tc.tile_pool(name="vp", bufs=2))
            qtp = es.enter_context(tc.tile_pool(name="qtp", bufs=1))
            wkp = es.enter_context(tc.tile_pool(name="wkp", bufs=1))
            wvp = es.enter_context(tc.tile_pool(name="wvp", bufs=1))
            wqp = es.enter_context(tc.tile_pool(name="wqp", bufs=1))
            ptp = es.enter_context(tc.tile_pool(name="ptp", bufs=4))
            sqp = es.enter_context(tc.tile_pool(name="sqp", bufs=8))
            tbp = es.enter_context(tc.tile_pool(name="tbp", bufs=2))
            biasp = es.enter_context(tc.tile_pool(name="biasp", bufs=2))
            smalls = es.enter_context(tc.tile_pool(name="smalls", bufs=1))
            rmp = es.enter_context(tc.tile_pool(name="rmp", bufs=1))
            tqp = es.enter_context(tc.tile_pool(name="tqp", bufs=1))
            recipp = es.enter_context(tc.tile_pool(name="recipp", bufs=2))
            bcp = es.enter_context(tc.tile_pool(name="bcp", bufs=2))
            stgp = es.enter_context(tc.tile_pool(name="stgp", bufs=2))
            proj_ps = es.enter_context(
                tc.tile_pool(name="proj_ps", bufs=2, space="PSUM"))
            sc_ps = es.enter_context(
                tc.tile_pool(name="sc_ps", bufs=2, space="PSUM"))
            avA_ps = es.enter_context(
                tc.tile_pool(name="avA_ps", bufs=1, space="PSUM"))
            avB_ps = es.enter_context(
                tc.tile_pool(name="avB_ps", bufs=1, space="PSUM"))
            # ---- persistent tiles; startup DMAs chunked so the first
            # K-projection matmuls can begin after ~2 d-tiles arrive ----
            enc_sb = persist.tile([128, DT, TK], BF16, tag="enc")
            encr = enct.rearrange("(j p) t -> p j t", p=128)
            wk_sb = wkp.tile([128, DT, D], BF16, tag="wk")
            wkr0 = wkd[0].rearrange("(j p) o -> p j o", p=128)
            for k in range(DT):
                nc.sync.dma_start(out=enc_sb[:, k, :], in_=encr[:, k, :])
                nc.sync.dma_start(out=wk_sb[:, k, :], in_=wkr0[:, k, :])
            cval_sb = persist.tile([128, KT], F32, tag="cval")
            nc.sync.dma_start(out=cval_sb[:], in_=cvald[:])
            ones1 = persist.tile([128, 1], BF16, tag="ones1")
            nc.vector.memset(ones1[:], 1.0)
            bvf_sb = persist.tile([128, DT], F32, tag="bvf")
            nc.sync.dma_start(out=bvf_sb[:], in_=bvfd[:])

            xb = [xp.tile([128, DT, TQ], BF16, tag="x", name=f"x{j}")
                  for j in range(2)]
            nc.sync.dma_start(
                out=xb[0][:], in_=xt0.rearrange("(j p) t -> p j t", p=128)
            )
            ktb = [ktp.tile([128, DT, TK], BF16, tag="kt", name=f"kt{j}")
                   for j in range(2)]
            vb = [vp.tile([128, KT, H, HD + 1], BF16, tag="vaug", name=f"v{j}")
                  for j in range(2)]
            qt_sb = qtp.tile([128, DT, TQ], BF16, tag="qt")
            qraw_sb = qtp.tile([128, DT, TQ], BF16, tag="qraw")
            # scaled-ones column of v_aug (den weights): exp(mask) per token
            for j in range(2):
                for h in range(H):
                    nc.vector.tensor_copy(vb[j][:, :, h, HD : HD + 1], cval_sb[:])

            wv_sb = wvp.tile([128, DT, D], BF16, tag="wv")
            wq_sb = wqp.tile([128, DT, D], BF16, tag="wq")

            def dma_w(dst, wd, i):
                wr = wd[i].rearrange("(j p) o -> p j o", p=128)
                for k in range(DT):
                    nc.sync.dma_start(out=dst[:, k, :], in_=wr[:, k, :])

            bc_sb = [None, None]

            def dma_consts(i):
                b = biasp.tile([128, 42], F32, tag="bcons")
                nc.sync.dma_start(out=b[:], in_=bcons[i])
                bc_sb[i % 2] = b

            # ---- fill generator: K+V projections for layer i ----
            def kv_gen(i):
                kt_dst = ktb[i % 2]
                v_dst = vb[i % 2]
                bcs = bc_sb[i % 2]
                for half in range(2):
                    for nl in range(4):
                        n = half * 4 + nl
                        for c in range(2):
                            ps = proj_ps.tile([128, 512], F32, tag="proj")
                            for k in range(DT):
                                nc.tensor.matmul(
                                    ps[:],
                                    wk_sb[:, k, ts(half, 512)][:, ts(nl, 128)],
                                    enc_sb[:, k, ts(c, 512)],
                                    start=(k == 0), stop=(k == DT - 1),
                                )
                                yield 1
                            nc.scalar.add(
                                kt_dst[:, n, ts(c, 512)], ps[:],
                                bcs[:, n : n + 1],
                            )
                    for m in range(KT):
                        ps = proj_ps.tile([128, 512], F32, tag="proj")
                        for k in range(DT):
                            nc.tensor.matmul(
                                ps[:], enc_sb[:, k, ts(m, 128)],
                                wv_sb[:, k, ts(half, 512)],
                                start=(k == 0), stop=(k == DT - 1),
                            )
                            yield 1
                        # scale token rows by exp(mask) while copying
                        if CVAL_ONES:
                            nc.scalar.copy(
                                v_dst[:, m, ts(half, 8), 0:HD],
                                ps[:].rearrange("p (h e) -> p h e", h=8),
                            )
                        else:
                            nc.vector.tensor_scalar_mul(
                                v_dst[:, m, ts(half, 8), 0:HD],
                                ps[:].rearrange("p (h e) -> p h e", h=8),
                                cval_sb[:, m : m + 1],
                            )

            _DONE = object()

            def pull(g, n):
                for _ in range(n):
                    if next(g, _DONE) is _DONE:
                        return False
                return True

            def drain(g):
                while next(g, _DONE) is not _DONE:
                    pass

            # ---- LN stats + smalls + Q projection + fixup for layer i ----
            def ln_q(i, sq_tiles):
                x = xb[i % 2]
                bcs = bc_sb[i % 2]
                lnt = proj_ps.tile([128, 512], F32, tag="proj")
                for k in range(DT):
                    nc.tensor.matmul(
                        lnt[0:1, :], ones1[:], x[:, k, :],
                        start=(k == 0), stop=(k == DT - 1),
                    )
                for k in range(DT):
                    nc.tensor.matmul(
                        lnt[32:33, :], ones1[:], sq_tiles[k][:],
                        start=(k == 0), stop=(k == DT - 1),
                    )
                # smalls: mu, var, rstd (fast inverse sqrt), M = -mu*rstd
                mu = smalls.tile([1, TQ], F32, tag="mu")
                nc.vector.tensor_scalar(
                    mu[:], lnt[0:1, :], bcs[0:1, 32:33], 1.0 / D,
                    op0=OP.add, op1=OP.mult,
                )
                e2 = smalls.tile([1, TQ], F32, tag="e2")
                nc.vector.tensor_scalar(
                    e2[:], lnt[32:33, :], bcs[0:1, 33:34], 1.0 / D,
                    op0=OP.add, op1=OP.mult,
                )
                msq = smalls.tile([1, TQ], F32, tag="msq")
                nc.vector.tensor_mul(msq[:], mu[:], mu[:])
                # e2 <- var = e2 - mu^2   (EPS folded into bcs[33] host-side)
                nc.vector.scalar_tensor_tensor(
                    e2[:], msq[:], -1.0, e2[:], op0=OP.mult, op1=OP.add
                )
                # fast inverse sqrt: y = bitcast(MAGIC - (v_int >> 1))
                y = smalls.tile([1, TQ], F32, tag="y")
                nc.vector.tensor_scalar(
                    y[:].bitcast(I32), e2[:].bitcast(I32), 1, -1,
                    op0=OP.logical_shift_right, op1=OP.bitwise_xor,
                )
                nc.vector.tensor_scalar_add(
                    y[:].bitcast(I32), y[:].bitcast(I32), RSQRT_MAGIC + 1
                )
                hv = smalls.tile([1, TQ], F32, tag="hv")
                nc.vector.tensor_scalar_mul(hv[:], e2[:], -0.5)
                u = smalls.tile([1, TQ], F32, tag="u")
                for _ in range(1):  # one Newton step (seed err 3.4% -> 0.17%)
                    nc.vector.tensor_mul(u[:], y[:], y[:])
                    nc.vector.tensor_mul(u[:], u[:], hv[:])
                    nc.vector.tensor_scalar_add(u[:], u[:], 1.5)
                    nc.vector.tensor_mul(y[:], y[:], u[:])
                m1 = smalls.tile([1, TQ], F32, tag="m1")
                nc.vector.scalar_tensor_tensor(
                    m1[:], mu[:], -1.0, y[:], op0=OP.mult, op1=OP.mult
                )
                R = rmp.tile([128, TQ], F32, tag="R")
                nc.gpsimd.partition_broadcast(R[:], y[:])
                M = rmp.tile([128, TQ], F32, tag="M")
                nc.gpsimd.partition_broadcast(M[:], m1[:])
                if DEBUG and i == 0:
                    nc.sync.dma_start(out=dbg_R[0:1, :], in_=y[:])
                    nc.sync.dma_start(out=dbg_R[1:2, :], in_=m1[:])

                # Q projection on raw x; consumer is a plain psum->sbuf
                # copy so the proj-psum ring never waits on the LN chain.
                for half in range(2):
                    for nl in range(4):
                        n = half * 4 + nl
                        qp = proj_ps.tile([128, 512], F32, tag="proj")
                        for k in range(DT):
                            nc.tensor.matmul(
                                qp[:],
                                wq_sb[:, k, ts(half, 512)][:, ts(nl, 128)],
                                x[:, k, :],
                                start=(k == 0), stop=(k == DT - 1),
                            )
                        nc.vector.tensor_copy(qraw_sb[:, n, :], qp[:])
                # deferred per-token fixup: q = R*(qraw + c) + M*s (+ bq)
                for n in range(DT):
                    tq = tqp.tile([128, TQ], F32, tag="tq")
                    nc.vector.scalar_tensor_tensor(
                        tq[:], qraw_sb[:, n, :], bcs[:, 8 + n : 9 + n], R[:],
                        op0=OP.add, op1=OP.mult,
                    )
                    nc.vector.scalar_tensor_tensor(
                        qt_sb[:, n, :], M[:], bcs[:, 16 + n : 17 + n],
                        tq[:], op0=OP.mult, op1=OP.add,
                    )
                    if not SKIP_BQ:
                        nc.vector.tensor_scalar_add(
                            qt_sb[:, n, :], qt_sb[:, n, :],
                            bcs[:, 24 + n : 25 + n],
                        )

            # ---- attention for layer i, pulling fill work between groups ----
            def attention(i, fill):
                kt_sb = ktb[i % 2]
                vaug = vb[i % 2]
                last = i == L - 1
                x_next = None if last else xb[(i + 1) % 2]
                sq_tiles = []
                for hp in range(DT):
                    hA, hB = 2 * hp, 2 * hp + 1
                    avA = avA_ps.tile([HD + 1, TQ], F32, tag="avA")
                    avB = avB_ps.tile([HD + 1, TQ], F32, tag="avB")
                    pts = []

                    def av_one(kk):
                        pt = pts[kk]
                        nc.tensor.matmul(
                            avA[:], vaug[:, kk, hA, :], pt[:, 0, :],
                            start=(kk == 0), stop=(kk == KT - 1),
                        )
                        nc.tensor.matmul(
                            avB[:], vaug[:, kk, hB, :], pt[:, 1, :],
                            start=(kk == 0), stop=(kk == KT - 1),
                        )

                    for ktp in range(4):
                        scs = []
                        for sub in range(2):
                            kk = 2 * ktp + sub
                            sc = sc_ps.tile([128, 2, TQ], F32, tag="sc")
                            nc.tensor.matmul(
                                sc[:, 0, :],
                                kt_sb[0:64, hp, ts(kk, 128)],
                                qt_sb[0:64, hp, :],
                                start=True, stop=True,
                            )
                            nc.tensor.matmul(
                                sc[:, 1, :],
                                kt_sb[64:128, hp, ts(kk, 128)],
                                qt_sb[64:128, hp, :],
                                start=True, stop=True,
                            )
                            scs.append(sc)
                        for sub in range(2):
                            pt = ptp.tile([128, 2, TQ], BF16, tag="pt")
                            nc.scalar.activation(pt[:], scs[sub][:], AF.Exp)
                            pts.append(pt)
                        if ktp > 0:
                            av_one(2 * ktp - 2)
                            av_one(2 * ktp - 1)
                        pull(fill, 5)
                    av_one(KT - 2)
                    av_one(KT - 1)

                    # normalize: x[d,t] = av[d,t] / den[t]
                    st = stgp.tile([128, TQ], F32, tag="stage", name="st") if last else None
                    for av, o in ((avA, 0), (avB, 64)):
                        r0 = recipp.tile([1, TQ], F32, tag="r0")
                        nc.vector.tensor_copy(r0[:], av[HD : HD + 1, :])
                        nc.vector.reciprocal_approx_fast(r0[:], r0[:])
                        bc = bcp.tile([64, TQ], F32, tag="bc")
                        nc.gpsimd.partition_broadcast(bc[:], r0[:])
                        if not last:
                            nc.vector.tensor_tensor(
                                x_next[o : o + 64, hp, :], av[0:HD, :], bc[:],
                                op=OP.mult,
                            )
                        else:
                            nc.vector.tensor_tensor(
                                st[o : o + 64, :], av[0:HD, :], bc[:],
                                op=OP.mult,
                            )
                            nc.vector.tensor_scalar_add(
                                st[o : o + 64, :], st[o : o + 64, :],
                                bvf_sb[o : o + 64, hp : hp + 1],
                            )
                    if last:
                        nc.sync.dma_start(out=outr[:, hp, :], in_=st[:])
                    if not last:
                        sqt = sqp.tile([128, TQ], BF16, tag="sq")
                        if SKIP_BVP:
                            nc.vector.tensor_mul(
                                sqt[:], x_next[:, hp, :], x_next[:, hp, :]
                            )
                        else:
                            bcn = bc_sb[(i + 1) % 2]
                            tb = tbp.tile([128, TQ], BF16, tag="tb")
                            nc.vector.tensor_scalar_add(
                                tb[:], x_next[:, hp, :],
                                bcn[:, 34 + hp : 35 + hp],
                            )
                            nc.vector.tensor_mul(sqt[:], tb[:], tb[:])
                        sq_tiles.append(sqt)
                return sq_tiles

            def dbg_dump(dst, src_tile):
                nc.sync.dma_start(out=dst, in_=src_tile[:])

            # ================= schedule =================
            # prologue: layer 0 projections + LN + Q
            dma_consts(0)
            dma_w(wv_sb, wvd, 0)
            dma_w(wq_sb, wqd, 0)
            g0 = kv_gen(0)
            drain(g0)
            sq0 = []
            for k in range(DT):
                sqt = sqp.tile([128, TQ], BF16, tag="sq")
                if SKIP_BVP:
                    nc.vector.tensor_mul(
                        sqt[:], xb[0][:, k, :], xb[0][:, k, :]
                    )
                else:
                    tb = tbp.tile([128, TQ], BF16, tag="tb")
                    nc.vector.tensor_scalar_add(
                        tb[:], xb[0][:, k, :], bc_sb[0][:, 34 + k : 35 + k]
                    )
                    nc.vector.tensor_mul(sqt[:], tb[:], tb[:])
                sq0.append(sqt)
            ln_q(0, sq0)

            if DEBUG:
                dbg_dump(dbg_kt, ktb[0])
                dbg_dump(dbg_v, vb[0])
                dbg_dump(dbg_qt, qt_sb)
            for i in range(L):
                if i < L - 1:
                    dma_consts(i + 1)
                    dma_w(wk_sb, wkd, i + 1)
                    dma_w(wv_sb, wvd, i + 1)
                    dma_w(wq_sb, wqd, i + 1)
                    fill = kv_gen(i + 1)
                else:
                    fill = iter(())
                sq_tiles = attention(i, fill)
                if DEBUG and i == 0:
                    dbg_dump(dbg_x1, xb[1])
                if i < L - 1:
                    drain(fill)
                    ln_q(i + 1, sq_tiles)

    nc.compile()
    _PROGRAM[spec] = nc
    return nc


def _stage_inputs(input_ids, encoder_state, cross_attn_mask, emb,
                  ln_g, ln_b, wq, bq, wk, bk, wv, bv):
    input_ids = np.asarray(input_ids)
    emb = np.asarray(emb, dtype=np.float32)
    encoder_state = np.asarray(encoder_state, dtype=np.float32)
    cross_attn_mask = np.asarray(cross_attn_mask, dtype=np.float32)
    ln_g = np.asarray(ln_g, dtype=np.float32)
    ln_b = np.asarray(ln_b, dtype=np.float32)
    wq = np.asarray(wq, dtype=np.float32)
    bq = np.asarray(bq, dtype=np.float32)
    wk = np.asarray(wk, dtype=np.float32)
    bk = np.asarray(bk, dtype=np.float32)
    wv = np.asarray(wv, dtype=np.float32)
    bv = np.asarray(bv, dtype=np.float32)

    scale = 1.0 / np.sqrt(HD)
    wq2 = ln_g[:, :, None] * wq * scale  # [L, D, D]
    bq2 = (np.einsum("ld,lde->le", ln_b, wq) + bq) * scale  # [L, D]
    bv_prev = np.concatenate([np.zeros((1, D), np.float32), bv[:-1]], axis=0)
    cq = np.einsum("lde,ld->le", wq2, bv_prev)  # [L, D]
    sq_ = wq2.sum(axis=1)  # [L, D]

    def pcol(a):  # [L, D] -> [L, 128, DT]
        return np.ascontiguousarray(a.reshape(-1, DT, 128).transpose(0, 2, 1))

    bcons = np.zeros((L, 128, 42), np.float32)
    bcons[:, :, 0:8] = pcol(bk)
    bcons[:, :, 8:16] = pcol(cq)
    bcons[:, :, 16:24] = pcol(sq_)
    bcons[:, :, 24:32] = pcol(bq2)
    bcons[:, 0, 32] = bv_prev.sum(axis=1)
    bcons[:, 0, 33] = D * EPS
    bcons[:, :, 34:42] = pcol(bv_prev)
    bf = ml_dtypes.bfloat16

    shared = {
        "wqd": wq2.astype(bf),
        "wkd": wk.astype(bf),
        "wvd": wv.astype(bf),
        "bcons": np.ascontiguousarray(bcons),
        "bvfd": np.ascontiguousarray(pcol(bv[L - 1 : L])[0]),
    }

    x0 = emb[input_ids]  # [NB, TQ, D]
    in_maps = []
    for n in range(NB):
        m = dict(shared)
        m["xt0"] = np.ascontiguousarray(x0[n].T).astype(bf)
        m["enct"] = np.ascontiguousarray(encoder_state[n].T).astype(bf)
        mk = cross_attn_mask[n, 0, 0]  # [TK]
        m["cvald"] = np.ascontiguousarray(
            np.exp(mk).reshape(KT, 128).T.astype(np.float32)
        )
        in_maps.append(m)
    return in_maps


def kernel(**inputs) -> np.ndarray:
    ln_b = np.asarray(inputs["ln_b"], np.float32)
    bq = np.asarray(inputs["bq"], np.float32)
    bv = np.asarray(inputs["bv"], np.float32)
    mask = np.asarray(inputs["cross_attn_mask"], np.float32)
    spec = (
        bool(not ln_b.any() and not bq.any()),
        bool(not bv[:-1].any()),
        bool(not mask.any()),
    )
    nc = build_program(spec)
    in_maps = _stage_inputs(**inputs)
    res = run_bass_kernel_spmd(nc, in_maps, list(range(NB)))
    out = np.stack([np.asarray(res.results[n]["outd"]).T for n in range(NB)])
    return np.ascontiguousarray(out, dtype=np.float32)


if __name__ == "__main__":
    build_program()
    print("program built ok")


# revision 19
# speedup vs baseline: 1.0474x; 1.0305x over previous
"""Trainium2 Bass kernel for nn_DecoderPreLN2 (4-layer cross-attention decoder).

Sharding: data-parallel over batch N=8 across 8 NeuronCores (1 element/core).
Per-core dataflow is fully "transposed" (D-major): activations live as x^T
[D=1024 partitions(8 tiles), T free] so every matmul keeps weights stationary
and no on-device transposes are needed.

v2 design (vs fp32r baseline):
  - All matmuls bf16 (measured same row-rate as fp32r but lighter LDWEIGHTS
    and half the DMA); PSUM accumulates f32.
  - The PE is kept *continuously busy* so the HAM clock gate stays at 8/8
    (2.4 GHz): K/V projections of layer i+1 are interleaved as "fill" matmuls
    into the ACT-gated gaps of attention(i).
  - exp(mask) is folded into the V ones-column (host computes exp(mask);
    the V-copy scales rows by it), so the score exp needs no bias operand
    and two k-tiles are merged per ACTIVATE (bf16 out).
  - LN affine + 1/sqrt(HD) folded into wq (host). LN statistics are computed
    on raw attention output via PE ones-matmuls into a proj-pool psum tile;
    the per-token LN scalars are applied as a *post*-fixup on the Q
    projection output (q = R*(qraw + c) + M*s + bq), so Q matmuls never wait
    on the LN chain.
  - V-bias bv is folded into the next layer's LN stats (softmax weights sum
    to 1); the last layer adds bv explicitly.
  - rstd via DVE-only fast-inverse-sqrt (bit trick + 2 Newton steps): no
    ACT table switching (ACT only ever loads the exp set).
  - Softmax denominator via the (scaled) ones-column of V; normalize =
    DVE reciprocal + GPSIMD partition_broadcast + DVE multiply.
  - Last layer streams the f32 output to HBM per head-pair tile.
"""

import os
import sys

sys.path.insert(0, "/opt/trn_rl_repo")

DEBUG = bool(int(os.environ.get("KERNEL_DEBUG", "0")))

import numpy as np
import ml_dtypes

import concourse.bass as bass
import concourse.tile as tile
from concourse import bacc, mybir
from concourse.bass import ts
from concourse.bass_utils import run_bass_kernel_spmd

L, D, H, HD = 4, 1024, 16, 64
TQ, TK, NB = 512, 1024, 8
DT = D // 128  # 8 d-tiles
KT = TK // 128  # 8 k-token tiles
EPS = 1e-5

F32 = mybir.dt.float32
BF16 = mybir.dt.bfloat16
I32 = mybir.dt.int32
AF = mybir.ActivationFunctionType
OP = mybir.AluOpType

RSQRT_MAGIC = 0x5F3759DF

_PROGRAM = {}


def build_program(spec=(True, True, True)):
    """spec = (skip_bq, skip_bvp, cval_ones) input-derived specializations."""
    global _PROGRAM
    if spec in _PROGRAM:
        return _PROGRAM[spec]
    SKIP_BQ, SKIP_BVP, CVAL_ONES = spec

    nc = bacc.Bacc(
        "TRN2", target_bir_lowering=False, debug=False,
        dynamic_dma_scratch_size=2048,
    )

    xt0 = nc.dram_tensor("xt0", [D, TQ], BF16, kind="ExternalInput").ap()
    enct = nc.dram_tensor("enct", [D, TK], BF16, kind="ExternalInput").ap()
    cvald = nc.dram_tensor("cvald", [128, KT], F32, kind="ExternalInput").ap()
    wqd = nc.dram_tensor("wqd", [L, D, D], BF16, kind="ExternalInput").ap()
    wkd = nc.dram_tensor("wkd", [L, D, D], BF16, kind="ExternalInput").ap()
    wvd = nc.dram_tensor("wvd", [L, D, D], BF16, kind="ExternalInput").ap()
    # per-layer f32 columns: 0-7 bk, 8-15 cq, 16-23 sq, 24-31 bq2,
    # [0,32]=sum(bvp), [0,33]=D*EPS, 34-41 bvp (pcol)
    bcons = nc.dram_tensor("bcons", [L, 128, 42], F32, kind="ExternalInput").ap()
    bvfd = nc.dram_tensor("bvfd", [128, DT], F32, kind="ExternalInput").ap()
    outd = nc.dram_tensor("outd", [D, TQ], F32, kind="ExternalOutput").ap()
    outr = outd.rearrange("(j p) t -> p j t", p=128)
    if DEBUG:
        dbg_kt = nc.dram_tensor("dbg_kt", [128, DT, TK], BF16, kind="ExternalOutput").ap()
        dbg_v = nc.dram_tensor("dbg_v", [128, KT, H, HD + 1], BF16, kind="ExternalOutput").ap()
        dbg_qt = nc.dram_tensor("dbg_qt", [128, DT, TQ], BF16, kind="ExternalOutput").ap()
        dbg_R = nc.dram_tensor("dbg_R", [2, TQ], F32, kind="ExternalOutput").ap()
        dbg_x1 = nc.dram_tensor("dbg_x1", [128, DT, TQ], BF16, kind="ExternalOutput").ap()
        dbg_pt = nc.dram_tensor("dbg_pt", [128, 2, 2, TQ], BF16, kind="ExternalOutput").ap()

    from contextlib import ExitStack

    with tile.TileContext(nc) as tc, ExitStack() as es:
            persist = es.enter_context(tc.tile_pool(name="persist", bufs=1))
            xp = es.enter_context(tc.tile_pool(name="xp", bufs=2))
            ktp = es.enter_context(tc.tile_pool(name="ktp", bufs=2))
            vp = es.enter_context(tc.tile_pool(name="vp", bufs=2))
            qtp = es.enter_context(tc.tile_pool(name="qtp", bufs=1))
            wkp = es.enter_context(tc.tile_pool(name="wkp", bufs=1))
            wvp = es.enter_context(tc.tile_pool(name="wvp", bufs=1))
            wqp = es.enter_context(tc.tile_pool(name="wqp", bufs=1))
            ptp = es.enter_context(tc.tile_pool(name="ptp", bufs=4))
            sqp = es.enter_context(tc.tile_pool(name="sqp", bufs=8))
            tbp = es.enter_context(tc.tile_pool(name="tbp", bufs=2))
            biasp = es.enter_context(tc.tile_pool(name="biasp", bufs=2))
            smalls = es.enter_context(tc.tile_pool(name="smalls", bufs=1))
            rmp = es.enter_context(tc.tile_pool(name="rmp", bufs=1))
            tqp = es.enter_context(tc.tile_pool(name="tqp", bufs=1))
            recipp = es.enter_context(tc.tile_pool(name="recipp", bufs=2))
            bcp = es.enter_context(tc.tile_pool(name="bcp", bufs=2))
            stgp = es.enter_context(tc.tile_pool(name="stgp", bufs=2))
            proj_ps = es.enter_context(
                tc.tile_pool(name="proj_ps", bufs=2, space="PSUM"))
            sc_ps = es.enter_context(
                tc.tile_pool(name="sc_ps", bufs=2, space="PSUM"))
            avA_ps = es.enter_context(
                tc.tile_pool(name="avA_ps", bufs=1, space="PSUM"))
            avB_ps = es.enter_context(
                tc.tile_pool(name="avB_ps", bufs=1, space="PSUM"))
            # ---- persistent tiles; startup DMAs chunked so the first
            # K-projection matmuls can begin after ~2 d-tiles arrive ----
            enc_sb = persist.tile([128, DT, TK], BF16, tag="enc")
            encr = enct.rearrange("(j p) t -> p j t", p=128)
            wk_sb = wkp.tile([128, DT, D], BF16, tag="wk")
            wkr0 = wkd[0].rearrange("(j p) o -> p j o", p=128)
            for k in range(DT):
                nc.sync.dma_start(out=enc_sb[:, k, :], in_=encr[:, k, :])
                nc.sync.dma_start(out=wk_sb[:, k, :], in_=wkr0[:, k, :])
            cval_sb = persist.tile([128, KT], F32, tag="cval")
            nc.sync.dma_start(out=cval_sb[:], in_=cvald[:])
            ones1 = persist.tile([128, 1], BF16, tag="ones1")
            nc.vector.memset(ones1[:], 1.0)
            bvf_sb = persist.tile([128, DT], F32, tag="bvf")
            nc.sync.dma_start(out=bvf_sb[:], in_=bvfd[:])

            xb = [xp.tile([128, DT, TQ], BF16, tag="x", name=f"x{j}")
                  for j in range(2)]
            nc.sync.dma_start(
                out=xb[0][:], in_=xt0.rearrange("(j p) t -> p j t", p=128)
            )
            ktb = [ktp.tile([128, DT, TK], BF16, tag="kt", name=f"kt{j}")
                   for j in range(2)]
            vb = [vp.tile([128, KT, H, HD + 1], BF16, tag="vaug", name=f"v{j}")
                  for j in range(2)]
            qt_sb = qtp.tile([128, DT, TQ], BF16, tag="qt")
            qraw_sb = qtp.tile([128, DT, TQ], BF16, tag="qraw")
            # scaled-ones column of v_aug (den weights): exp(mask) per token
            for j in range(2):
                for h in range(H):
                    nc.vector.tensor_copy(vb[j][:, :, h, HD : HD + 1], cval_sb[:])

            wv_sb = wvp.tile([128, DT, D], BF16, tag="wv")
            wq_sb = wqp.tile([128, DT, D], BF16, tag="wq")

            def dma_w(dst, wd, i):
                wr = wd[i].rearrange("(j p) o -> p j o", p=128)
                for k in range(DT):
                    nc.sync.dma_start(out=dst[:, k, :], in_=wr[:, k, :])

            bc_sb = [None, None]

            def dma_consts(i):
                b = biasp.tile([128, 42], F32, tag="bcons")
                nc.sync.dma_start(out=b[:], in_=bcons[i])
                bc_sb[i % 2] = b

            # ---- fill generator: K+V projections for layer i (one half:
            # heads [half*8, half*8+8) i.e. kt blocks n=half*4..half*4+3) ----
            def kv_gen(i, halves=(0, 1)):
                kt_dst = ktb[i % 2]
                v_dst = vb[i % 2]
                bcs = bc_sb[i % 2]
                for half in halves:
                    for nl in range(4):
                        n = half * 4 + nl
                        for c in range(2):
                            ps = proj_ps.tile([128, 512], F32, tag="proj")
                            for k in range(DT):
                                nc.tensor.matmul(
                                    ps[:],
                                    wk_sb[:, k, ts(half, 512)][:, ts(nl, 128)],
                                    enc_sb[:, k, ts(c, 512)],
                                    start=(k == 0), stop=(k == DT - 1),
                                )
                                yield 1
                            nc.scalar.add(
                                kt_dst[:, n, ts(c, 512)], ps[:],
                                bcs[:, n : n + 1],
                            )
                    for m in range(KT):
                        ps = proj_ps.tile([128, 512], F32, tag="proj")
                        for k in range(DT):
                            nc.tensor.matmul(
                                ps[:], enc_sb[:, k, ts(m, 128)],
                                wv_sb[:, k, ts(half, 512)],
                                start=(k == 0), stop=(k == DT - 1),
                            )
                            yield 1
                        # scale token rows by exp(mask) while copying
                        if CVAL_ONES:
                            nc.scalar.copy(
                                v_dst[:, m, ts(half, 8), 0:HD],
                                ps[:].rearrange("p (h e) -> p h e", h=8),
                            )
                        else:
                            nc.vector.tensor_scalar_mul(
                                v_dst[:, m, ts(half, 8), 0:HD],
                                ps[:].rearrange("p (h e) -> p h e", h=8),
                                cval_sb[:, m : m + 1],
                            )

            _DONE = object()

            def pull(gens, n):
                while n > 0 and gens:
                    if next(gens[0], _DONE) is _DONE:
                        gens.pop(0)
                    else:
                        n -= 1

            def drain(gens):
                while gens:
                    if next(gens[0], _DONE) is _DONE:
                        gens.pop(0)

            # ---- LN stats + smalls + Q projection + fixup for layer i ----
            def ln_q(i, sq_tiles):
                x = xb[i % 2]
                bcs = bc_sb[i % 2]
                lnt = proj_ps.tile([128, 512], F32, tag="proj")
                for k in range(DT):
                    nc.tensor.matmul(
                        lnt[0:1, :], ones1[:], x[:, k, :],
                        start=(k == 0), stop=(k == DT - 1),
                    )
                for k in range(DT):
                    nc.tensor.matmul(
                        lnt[32:33, :], ones1[:], sq_tiles[k][:],
                        start=(k == 0), stop=(k == DT - 1),
                    )
                # smalls: mu, var, rstd (fast inverse sqrt), M = -mu*rstd
                mu = smalls.tile([1, TQ], F32, tag="mu")
                nc.vector.tensor_scalar(
                    mu[:], lnt[0:1, :], bcs[0:1, 32:33], 1.0 / D,
                    op0=OP.add, op1=OP.mult,
                )
                e2 = smalls.tile([1, TQ], F32, tag="e2")
                nc.vector.tensor_scalar(
                    e2[:], lnt[32:33, :], bcs[0:1, 33:34], 1.0 / D,
                    op0=OP.add, op1=OP.mult,
                )
                msq = smalls.tile([1, TQ], F32, tag="msq")
                nc.vector.tensor_mul(msq[:], mu[:], mu[:])
                # e2 <- var = e2 - mu^2   (EPS folded into bcs[33] host-side)
                nc.vector.scalar_tensor_tensor(
                    e2[:], msq[:], -1.0, e2[:], op0=OP.mult, op1=OP.add
                )
                # fast inverse sqrt: y = bitcast(MAGIC - (v_int >> 1))
                y = smalls.tile([1, TQ], F32, tag="y")
                nc.vector.tensor_scalar(
                    y[:].bitcast(I32), e2[:].bitcast(I32), 1, -1,
                    op0=OP.logical_shift_right, op1=OP.bitwise_xor,
                )
                nc.vector.tensor_scalar_add(
                    y[:].bitcast(I32), y[:].bitcast(I32), RSQRT_MAGIC + 1
                )
                hv = smalls.tile([1, TQ], F32, tag="hv")
                nc.vector.tensor_scalar_mul(hv[:], e2[:], -0.5)
                u = smalls.tile([1, TQ], F32, tag="u")
                for _ in range(1):  # one Newton step (seed err 3.4% -> 0.17%)
                    nc.vector.tensor_mul(u[:], y[:], y[:])
                    nc.vector.tensor_mul(u[:], u[:], hv[:])
                    nc.vector.tensor_scalar_add(u[:], u[:], 1.5)
                    nc.vector.tensor_mul(y[:], y[:], u[:])
                m1 = smalls.tile([1, TQ], F32, tag="m1")
                nc.vector.scalar_tensor_tensor(
                    m1[:], mu[:], -1.0, y[:], op0=OP.mult, op1=OP.mult
                )
                R = rmp.tile([128, TQ], F32, tag="R")
                nc.gpsimd.partition_broadcast(R[:], y[:])
                M = rmp.tile([128, TQ], F32, tag="M")
                nc.gpsimd.partition_broadcast(M[:], m1[:])
                if DEBUG and i == 0:
                    nc.sync.dma_start(out=dbg_R[0:1, :], in_=y[:])
                    nc.sync.dma_start(out=dbg_R[1:2, :], in_=m1[:])

                # Q projection on raw x; consumer is a plain psum->sbuf
                # copy so the proj-psum ring never waits on the LN chain.
                for half in range(2):
                    for nl in range(4):
                        n = half * 4 + nl
                        qp = proj_ps.tile([128, 512], F32, tag="proj")
                        for k in range(DT):
                            nc.tensor.matmul(
                                qp[:],
                                wq_sb[:, k, ts(half, 512)][:, ts(nl, 128)],
                                x[:, k, :],
                                start=(k == 0), stop=(k == DT - 1),
                            )
                        nc.vector.tensor_copy(qraw_sb[:, n, :], qp[:])
                # deferred per-token fixup: q = R*(qraw + c) + M*s (+ bq)
                for n in range(DT):
                    tq = tqp.tile([128, TQ], F32, tag="tq")
                    nc.vector.scalar_tensor_tensor(
                        tq[:], qraw_sb[:, n, :], bcs[:, 8 + n : 9 + n], R[:],
                        op0=OP.add, op1=OP.mult,
                    )
                    nc.vector.scalar_tensor_tensor(
                        qt_sb[:, n, :], M[:], bcs[:, 16 + n : 17 + n],
                        tq[:], op0=OP.mult, op1=OP.add,
                    )
                    if not SKIP_BQ:
                        nc.vector.tensor_scalar_add(
                            qt_sb[:, n, :], qt_sb[:, n, :],
                            bcs[:, 24 + n : 25 + n],
                        )

            # ---- attention for layer i, pulling fill work between groups ----
            def attention(i, fill):
                kt_sb = ktb[i % 2]
                vaug = vb[i % 2]
                last = i == L - 1
                x_next = None if last else xb[(i + 1) % 2]
                sq_tiles = []
                for hp in range(DT):
                    if last and hp == DT // 2:
                        drain(fill)
                    hA, hB = 2 * hp, 2 * hp + 1
                    avA = avA_ps.tile([HD + 1, TQ], F32, tag="avA")
                    avB = avB_ps.tile([HD + 1, TQ], F32, tag="avB")
                    pts = []

                    def av_one(kk):
                        pt = pts[kk]
                        nc.tensor.matmul(
                            avA[:], vaug[:, kk, hA, :], pt[:, 0, :],
                            start=(kk == 0), stop=(kk == KT - 1),
                        )
                        nc.tensor.matmul(
                            avB[:], vaug[:, kk, hB, :], pt[:, 1, :],
                            start=(kk == 0), stop=(kk == KT - 1),
                        )

                    for ktp in range(4):
                        scs = []
                        for sub in range(2):
                            kk = 2 * ktp + sub
                            sc = sc_ps.tile([128, 2, TQ], F32, tag="sc")
                            nc.tensor.matmul(
                                sc[:, 0, :],
                                kt_sb[0:64, hp, ts(kk, 128)],
                                qt_sb[0:64, hp, :],
                                start=True, stop=True,
                            )
                            nc.tensor.matmul(
                                sc[:, 1, :],
                                kt_sb[64:128, hp, ts(kk, 128)],
                                qt_sb[64:128, hp, :],
                                start=True, stop=True,
                            )
                            scs.append(sc)
                        for sub in range(2):
                            pt = ptp.tile([128, 2, TQ], BF16, tag="pt")
                            nc.scalar.activation(pt[:], scs[sub][:], AF.Exp)
                            pts.append(pt)
                        if ktp > 0:
                            av_one(2 * ktp - 2)
                            av_one(2 * ktp - 1)
                        pull(fill, 5)
                    av_one(KT - 2)
                    av_one(KT - 1)

                    # normalize: x[d,t] = av[d,t] / den[t]
                    st = stgp.tile([128, TQ], F32, tag="stage", name="st") if last else None
                    for av, o in ((avA, 0), (avB, 64)):
                        r0 = recipp.tile([1, TQ], F32, tag="r0")
                        nc.vector.tensor_copy(r0[:], av[HD : HD + 1, :])
                        nc.vector.reciprocal_approx_fast(r0[:], r0[:])
                        bc = bcp.tile([64, TQ], F32, tag="bc")
                        nc.gpsimd.partition_broadcast(bc[:], r0[:])
                        if not last:
                            nc.vector.tensor_tensor(
                                x_next[o : o + 64, hp, :], av[0:HD, :], bc[:],
                                op=OP.mult,
                            )
                        else:
                            nc.vector.tensor_tensor(
                                st[o : o + 64, :], av[0:HD, :], bc[:],
                                op=OP.mult,
                            )
                            nc.vector.tensor_scalar_add(
                                st[o : o + 64, :], st[o : o + 64, :],
                                bvf_sb[o : o + 64, hp : hp + 1],
                            )
                    if last:
                        nc.sync.dma_start(out=outr[:, hp, :], in_=st[:])
                    if not last:
                        sqt = sqp.tile([128, TQ], BF16, tag="sq")
                        if SKIP_BVP:
                            nc.vector.tensor_mul(
                                sqt[:], x_next[:, hp, :], x_next[:, hp, :]
                            )
                        else:
                            bcn = bc_sb[(i + 1) % 2]
                            tb = tbp.tile([128, TQ], BF16, tag="tb")
                            nc.vector.tensor_scalar_add(
                                tb[:], x_next[:, hp, :],
                                bcn[:, 34 + hp : 35 + hp],
                            )
                            nc.vector.tensor_mul(sqt[:], tb[:], tb[:])
                        sq_tiles.append(sqt)
                return sq_tiles

            def dbg_dump(dst, src_tile):
                nc.sync.dma_start(out=dst, in_=src_tile[:])

            # ================= schedule =================
            # prologue: layer 0 projections + LN + Q
            dma_consts(0)
            dma_w(wv_sb, wvd, 0)
            dma_w(wq_sb, wqd, 0)
            drain([kv_gen(0)])
            sq0 = []
            for k in range(DT):
                sqt = sqp.tile([128, TQ], BF16, tag="sq")
                if SKIP_BVP:
                    nc.vector.tensor_mul(
                        sqt[:], xb[0][:, k, :], xb[0][:, k, :]
                    )
                else:
                    tb = tbp.tile([128, TQ], BF16, tag="tb")
                    nc.vector.tensor_scalar_add(
                        tb[:], xb[0][:, k, :], bc_sb[0][:, 34 + k : 35 + k]
                    )
                    nc.vector.tensor_mul(sqt[:], tb[:], tb[:])
                sq0.append(sqt)
            ln_q(0, sq0)

            if DEBUG:
                dbg_dump(dbg_kt, ktb[0])
                dbg_dump(dbg_v, vb[0])
                dbg_dump(dbg_qt, qt_sb)
            fill = []
            for i in range(L):
                if i < L - 1:
                    dma_consts(i + 1)
                    dma_w(wk_sb, wkd, i + 1)
                    dma_w(wv_sb, wvd, i + 1)
                    dma_w(wq_sb, wqd, i + 1)
                    fill = [kv_gen(i + 1, (0,)), kv_gen(i + 1, (1,))]
                sq_tiles = attention(i, fill)
                if DEBUG and i == 0:
                    dbg_dump(dbg_x1, xb[1])
                if i < L - 1:
                    if i == L - 2:
                        # leave half-1 of layer-3 K/V as fill work for
                        # attention(3) itself (its hp 0-3 don't need it)
                        if len(fill) == 2:
                            drain(fill[:1])
                            fill = fill[1:]
                        else:
                            pass
                    else:
                        drain(fill)
                        fill = []
                    ln_q(i + 1, sq_tiles)

    nc.compile()
    _PROGRAM[spec] = nc
    return nc


def _stage_inputs(input_ids, encoder_state, cross_attn_mask, emb,
                  ln_g, ln_b, wq, bq, wk, bk, wv, bv):
    input_ids = np.asarray(input_ids)
    emb = np.asarray(emb, dtype=np.float32)
    encoder_state = np.asarray(encoder_state, dtype=np.float32)
    cross_attn_mask = np.asarray(cross_attn_mask, dtype=np.float32)
    ln_g = np.asarray(ln_g, dtype=np.float32)
    ln_b = np.asarray(ln_b, dtype=np.float32)
    wq = np.asarray(wq, dtype=np.float32)
    bq = np.asarray(bq, dtype=np.float32)
    wk = np.asarray(wk, dtype=np.float32)
    bk = np.asarray(bk, dtype=np.float32)
    wv = np.asarray(wv, dtype=np.float32)
    bv = np.asarray(bv, dtype=np.float32)

    scale = 1.0 / np.sqrt(HD)
    wq2 = ln_g[:, :, None] * wq * scale  # [L, D, D]
    bq2 = (np.einsum("ld,lde->le", ln_b, wq) + bq) * scale  # [L, D]
    bv_prev = np.concatenate([np.zeros((1, D), np.float32), bv[:-1]], axis=0)
    cq = np.einsum("lde,ld->le", wq2, bv_prev)  # [L, D]
    sq_ = wq2.sum(axis=1)  # [L, D]

    def pcol(a):  # [L, D] -> [L, 128, DT]
        return np.ascontiguousarray(a.reshape(-1, DT, 128).transpose(0, 2, 1))

    bcons = np.zeros((L, 128, 42), np.float32)
    bcons[:, :, 0:8] = pcol(bk)
    bcons[:, :, 8:16] = pcol(cq)
    bcons[:, :, 16:24] = pcol(sq_)
    bcons[:, :, 24:32] = pcol(bq2)
    bcons[:, 0, 32] = bv_prev.sum(axis=1)
    bcons[:, 0, 33] = D * EPS
    bcons[:, :, 34:42] = pcol(bv_prev)
    bf = ml_dtypes.bfloat16

    shared = {
        "wqd": wq2.astype(bf),
        "wkd": wk.astype(bf),
        "wvd": wv.astype(bf),
        "bcons": np.ascontiguousarray(bcons),
        "bvfd": np.ascontiguousarray(pcol(bv[L - 1 : L])[0]),
    }

    x0 = emb[input_ids]  # [NB, TQ, D]
    in_maps = []
    for n in range(NB):
        m = dict(shared)
        m["xt0"] = np.ascontiguousarray(x0[n].T).astype(bf)
        m["enct"] = np.ascontiguousarray(encoder_state[n].T).astype(bf)
        mk = cross_attn_mask[n, 0, 0]  # [TK]
        m["cvald"] = np.ascontiguousarray(
            np.exp(mk).reshape(KT, 128).T.astype(np.float32)
        )
        in_maps.append(m)
    return in_maps


def kernel(**inputs) -> np.ndarray:
    ln_b = np.asarray(inputs["ln_b"], np.float32)
    bq = np.asarray(inputs["bq"], np.float32)
    bv = np.asarray(inputs["bv"], np.float32)
    mask = np.asarray(inputs["cross_attn_mask"], np.float32)
    spec = (
        bool(not ln_b.any() and not bq.any()),
        bool(not bv[:-1].any()),
        bool(not mask.any()),
    )
    nc = build_program(spec)
    in_maps = _stage_inputs(**inputs)
    res = run_bass_kernel_spmd(nc, in_maps, list(range(NB)))
    out = np.stack([np.asarray(res.results[n]["outd"]).T for n in range(NB)])
    return np.ascontiguousarray(out, dtype=np.float32)


if __name__ == "__main__":
    build_program()
    print("program built ok")


# revision 21
# speedup vs baseline: 1.0488x; 1.0013x over previous
"""Trainium2 Bass kernel for nn_DecoderPreLN2 (4-layer cross-attention decoder).

Sharding: data-parallel over batch N=8 across 8 NeuronCores (1 element/core).
Per-core dataflow is fully "transposed" (D-major): activations live as x^T
[D=1024 partitions(8 tiles), T free] so every matmul keeps weights stationary
and no on-device transposes are needed.

v2 design (vs fp32r baseline):
  - All matmuls bf16 (measured same row-rate as fp32r but lighter LDWEIGHTS
    and half the DMA); PSUM accumulates f32.
  - The PE is kept *continuously busy* so the HAM clock gate stays at 8/8
    (2.4 GHz): K/V projections of layer i+1 are interleaved as "fill" matmuls
    into the ACT-gated gaps of attention(i).
  - exp(mask) is folded into the V ones-column (host computes exp(mask);
    the V-copy scales rows by it), so the score exp needs no bias operand
    and two k-tiles are merged per ACTIVATE (bf16 out).
  - LN affine + 1/sqrt(HD) folded into wq (host). LN statistics are computed
    on raw attention output via PE ones-matmuls into a proj-pool psum tile;
    the per-token LN scalars are applied as a *post*-fixup on the Q
    projection output (q = R*(qraw + c) + M*s + bq), so Q matmuls never wait
    on the LN chain.
  - V-bias bv is folded into the next layer's LN stats (softmax weights sum
    to 1); the last layer adds bv explicitly.
  - rstd via DVE-only fast-inverse-sqrt (bit trick + 2 Newton steps): no
    ACT table switching (ACT only ever loads the exp set).
  - Softmax denominator via the (scaled) ones-column of V; normalize =
    DVE reciprocal + GPSIMD partition_broadcast + DVE multiply.
  - Last layer streams the f32 output to HBM per head-pair tile.
"""

import os
import sys

sys.path.insert(0, "/opt/trn_rl_repo")

DEBUG = bool(int(os.environ.get("KERNEL_DEBUG", "0")))

import numpy as np
import ml_dtypes

import concourse.bass as bass
import concourse.tile as tile
from concourse import bacc, mybir
from concourse.bass import ts
from concourse.bass_utils import run_bass_kernel_spmd

L, D, H, HD = 4, 1024, 16, 64
TQ, TK, NB = 512, 1024, 8
DT = D // 128  # 8 d-tiles
KT = TK // 128  # 8 k-token tiles
EPS = 1e-5

F32 = mybir.dt.float32
BF16 = mybir.dt.bfloat16
I32 = mybir.dt.int32
AF = mybir.ActivationFunctionType
OP = mybir.AluOpType

RSQRT_MAGIC = 0x5F3759DF

_PROGRAM = {}


def build_program(spec=(True, True, True)):
    """spec = (skip_bq, skip_bvp, cval_ones) input-derived specializations."""
    global _PROGRAM
    if spec in _PROGRAM:
        return _PROGRAM[spec]
    SKIP_BQ, SKIP_BVP, CVAL_ONES = spec

    nc = bacc.Bacc(
        "TRN2", target_bir_lowering=False, debug=False,
        dynamic_dma_scratch_size=2048,
    )

    xt0 = nc.dram_tensor("xt0", [D, TQ], BF16, kind="ExternalInput").ap()
    enct = nc.dram_tensor("enct", [D, TK], BF16, kind="ExternalInput").ap()
    cvald = nc.dram_tensor("cvald", [128, KT], F32, kind="ExternalInput").ap()
    wqd = nc.dram_tensor("wqd", [L, D, D], BF16, kind="ExternalInput").ap()
    wkd = nc.dram_tensor("wkd", [L, D, D], BF16, kind="ExternalInput").ap()
    wvd = nc.dram_tensor("wvd", [L, D, D], BF16, kind="ExternalInput").ap()
    # per-layer f32 columns: 0-7 bk, 8-15 cq, 16-23 sq, 24-31 bq2,
    # [0,32]=sum(bvp), [0,33]=D*EPS, 34-41 bvp (pcol)
    bcons = nc.dram_tensor("bcons", [L, 128, 42], F32, kind="ExternalInput").ap()
    bvfd = nc.dram_tensor("bvfd", [128, DT], F32, kind="ExternalInput").ap()
    outd = nc.dram_tensor("outd", [D, TQ], F32, kind="ExternalOutput").ap()
    outr = outd.rearrange("(j p) t -> p j t", p=128)
    if DEBUG:
        dbg_kt = nc.dram_tensor("dbg_kt", [128, DT, TK], BF16, kind="ExternalOutput").ap()
        dbg_v = nc.dram_tensor("dbg_v", [128, KT, H, HD + 1], BF16, kind="ExternalOutput").ap()
        dbg_qt = nc.dram_tensor("dbg_qt", [128, DT, TQ], BF16, kind="ExternalOutput").ap()
        dbg_R = nc.dram_tensor("dbg_R", [2, TQ], F32, kind="ExternalOutput").ap()
        dbg_x1 = nc.dram_tensor("dbg_x1", [128, DT, TQ], BF16, kind="ExternalOutput").ap()
        dbg_pt = nc.dram_tensor("dbg_pt", [128, 2, 2, TQ], BF16, kind="ExternalOutput").ap()

    from contextlib import ExitStack

    with tile.TileContext(nc) as tc, ExitStack() as es:
            persist = es.enter_context(tc.tile_pool(name="persist", bufs=1))
            xp = es.enter_context(tc.tile_pool(name="xp", bufs=2))
            ktp = es.enter_context(tc.tile_pool(name="ktp", bufs=2))
            vp = es.enter_context(tc.tile_pool(name="vp", bufs=2))
            qtp = es.enter_context(tc.tile_pool(name="qtp", bufs=1))
            wkp = es.enter_context(tc.tile_pool(name="wkp", bufs=1))
            wvp = es.enter_context(tc.tile_pool(name="wvp", bufs=1))
            wqp = es.enter_context(tc.tile_pool(name="wqp", bufs=1))
            ptp = es.enter_context(tc.tile_pool(name="ptp", bufs=4))
            sqp = es.enter_context(tc.tile_pool(name="sqp", bufs=8))
            tbp = es.enter_context(tc.tile_pool(name="tbp", bufs=2))
            biasp = es.enter_context(tc.tile_pool(name="biasp", bufs=2))
            smalls = es.enter_context(tc.tile_pool(name="smalls", bufs=1))
            rmp = es.enter_context(tc.tile_pool(name="rmp", bufs=1))
            tqp = es.enter_context(tc.tile_pool(name="tqp", bufs=1))
            recipp = es.enter_context(tc.tile_pool(name="recipp", bufs=2))
            bcp = es.enter_context(tc.tile_pool(name="bcp", bufs=2))
            stgp = es.enter_context(tc.tile_pool(name="stgp", bufs=2))
            proj_ps = es.enter_context(
                tc.tile_pool(name="proj_ps", bufs=2, space="PSUM"))
            sc_ps = es.enter_context(
                tc.tile_pool(name="sc_ps", bufs=2, space="PSUM"))
            avA_ps = es.enter_context(
                tc.tile_pool(name="avA_ps", bufs=1, space="PSUM"))
            avB_ps = es.enter_context(
                tc.tile_pool(name="avB_ps", bufs=1, space="PSUM"))
            # ---- persistent tiles; startup DMAs chunked so the first
            # K-projection matmuls can begin after ~2 d-tiles arrive ----
            enc_sb = persist.tile([128, DT, TK], BF16, tag="enc")
            encr = enct.rearrange("(j p) t -> p j t", p=128)
            wk_sb = wkp.tile([128, DT, D], BF16, tag="wk")
            wkr0 = wkd[0].rearrange("(j p) o -> p j o", p=128)
            for k in range(DT):
                nc.sync.dma_start(out=enc_sb[:, k, :], in_=encr[:, k, :])
                nc.sync.dma_start(out=wk_sb[:, k, :], in_=wkr0[:, k, :])
            cval_sb = persist.tile([128, KT], F32, tag="cval")
            nc.sync.dma_start(out=cval_sb[:], in_=cvald[:])
            ones1 = persist.tile([128, 1], BF16, tag="ones1")
            nc.vector.memset(ones1[:], 1.0)
            bvf_sb = persist.tile([128, DT], F32, tag="bvf")
            nc.sync.dma_start(out=bvf_sb[:], in_=bvfd[:])

            xb = [xp.tile([128, DT, TQ], BF16, tag="x", name=f"x{j}")
                  for j in range(2)]
            nc.sync.dma_start(
                out=xb[0][:], in_=xt0.rearrange("(j p) t -> p j t", p=128)
            )
            ktb = [ktp.tile([128, DT, TK], BF16, tag="kt", name=f"kt{j}")
                   for j in range(2)]
            vb = [vp.tile([128, KT, H, HD + 1], BF16, tag="vaug", name=f"v{j}")
                  for j in range(2)]
            qt_sb = qtp.tile([128, DT, TQ], BF16, tag="qt")
            qraw_sb = qtp.tile([128, DT, TQ], BF16, tag="qraw")
            # scaled-ones column of v_aug (den weights): exp(mask) per token
            for j in range(2):
                for h in range(H):
                    nc.vector.tensor_copy(vb[j][:, :, h, HD : HD + 1], cval_sb[:])

            wv_sb = wvp.tile([128, DT, D], BF16, tag="wv")
            wq_sb = wqp.tile([128, DT, D], BF16, tag="wq")

            def dma_w(dst, wd, i):
                wr = wd[i].rearrange("(j p) o -> p j o", p=128)
                for k in range(DT):
                    nc.sync.dma_start(out=dst[:, k, :], in_=wr[:, k, :])

            bc_sb = [None, None]

            def dma_consts(i):
                b = biasp.tile([128, 42], F32, tag="bcons")
                nc.sync.dma_start(out=b[:], in_=bcons[i])
                bc_sb[i % 2] = b

            # ---- fill generator: K+V projections for layer i (one half:
            # heads [half*8, half*8+8) i.e. kt blocks n=half*4..half*4+3) ----
            def kv_gen(i, halves=(0, 1)):
                kt_dst = ktb[i % 2]
                v_dst = vb[i % 2]
                bcs = bc_sb[i % 2]
                for half in halves:
                    for nl in range(4):
                        n = half * 4 + nl
                        for c in range(2):
                            ps = proj_ps.tile([128, 512], F32, tag="proj")
                            for k in range(DT):
                                nc.tensor.matmul(
                                    ps[:],
                                    wk_sb[:, k, ts(half, 512)][:, ts(nl, 128)],
                                    enc_sb[:, k, ts(c, 512)],
                                    start=(k == 0), stop=(k == DT - 1),
                                )
                                yield 1
                            nc.scalar.add(
                                kt_dst[:, n, ts(c, 512)], ps[:],
                                bcs[:, n : n + 1],
                            )
                    for m in range(KT):
                        ps = proj_ps.tile([128, 512], F32, tag="proj")
                        for k in range(DT):
                            nc.tensor.matmul(
                                ps[:], enc_sb[:, k, ts(m, 128)],
                                wv_sb[:, k, ts(half, 512)],
                                start=(k == 0), stop=(k == DT - 1),
                            )
                            yield 1
                        # scale token rows by exp(mask) while copying
                        if CVAL_ONES:
                            nc.scalar.copy(
                                v_dst[:, m, ts(half, 8), 0:HD],
                                ps[:].rearrange("p (h e) -> p h e", h=8),
                            )
                        else:
                            nc.vector.tensor_scalar_mul(
                                v_dst[:, m, ts(half, 8), 0:HD],
                                ps[:].rearrange("p (h e) -> p h e", h=8),
                                cval_sb[:, m : m + 1],
                            )

            _DONE = object()

            def pull(gens, n):
                while n > 0 and gens:
                    if next(gens[0], _DONE) is _DONE:
                        gens.pop(0)
                    else:
                        n -= 1

            def drain(gens):
                while gens:
                    if next(gens[0], _DONE) is _DONE:
                        gens.pop(0)

            # ---- LN stats + smalls + Q projection + fixup for layer i ----
            def ln_q(i, sq_tiles):
                x = xb[i % 2]
                bcs = bc_sb[i % 2]
                lnt = proj_ps.tile([128, 512], F32, tag="proj")
                for k in range(DT):
                    nc.tensor.matmul(
                        lnt[0:1, :], ones1[:], x[:, k, :],
                        start=(k == 0), stop=(k == DT - 1),
                    )
                for k in range(DT):
                    nc.tensor.matmul(
                        lnt[32:33, :], ones1[:], sq_tiles[k][:],
                        start=(k == 0), stop=(k == DT - 1),
                    )
                # smalls: mu, var, rstd (fast inverse sqrt), M = -mu*rstd
                mu = smalls.tile([1, TQ], F32, tag="mu")
                nc.vector.tensor_scalar(
                    mu[:], lnt[0:1, :], bcs[0:1, 32:33], 1.0 / D,
                    op0=OP.add, op1=OP.mult,
                )
                e2 = smalls.tile([1, TQ], F32, tag="e2")
                nc.vector.tensor_scalar(
                    e2[:], lnt[32:33, :], bcs[0:1, 33:34], 1.0 / D,
                    op0=OP.add, op1=OP.mult,
                )
                msq = smalls.tile([1, TQ], F32, tag="msq")
                nc.vector.tensor_mul(msq[:], mu[:], mu[:])
                # e2 <- var = e2 - mu^2   (EPS folded into bcs[33] host-side)
                nc.vector.scalar_tensor_tensor(
                    e2[:], msq[:], -1.0, e2[:], op0=OP.mult, op1=OP.add
                )
                # fast inverse sqrt: y = bitcast(MAGIC - (v_int >> 1))
                y = smalls.tile([1, TQ], F32, tag="y")
                nc.vector.tensor_scalar(
                    y[:].bitcast(I32), e2[:].bitcast(I32), 1, -1,
                    op0=OP.logical_shift_right, op1=OP.bitwise_xor,
                )
                nc.vector.tensor_scalar_add(
                    y[:].bitcast(I32), y[:].bitcast(I32), RSQRT_MAGIC + 1
                )
                hv = smalls.tile([1, TQ], F32, tag="hv")
                nc.vector.tensor_scalar_mul(hv[:], e2[:], -0.5)
                u = smalls.tile([1, TQ], F32, tag="u")
                for _ in range(1):  # one Newton step (seed err 3.4% -> 0.17%)
                    nc.vector.tensor_mul(u[:], y[:], y[:])
                    nc.vector.tensor_mul(u[:], u[:], hv[:])
                    nc.vector.tensor_scalar_add(u[:], u[:], 1.5)
                    nc.vector.tensor_mul(y[:], y[:], u[:])
                m1 = smalls.tile([1, TQ], F32, tag="m1")
                nc.vector.scalar_tensor_tensor(
                    m1[:], mu[:], -1.0, y[:], op0=OP.mult, op1=OP.mult
                )
                R = rmp.tile([128, TQ], F32, tag="R")
                nc.gpsimd.partition_broadcast(R[:], y[:])
                M = rmp.tile([128, TQ], F32, tag="M")
                nc.gpsimd.partition_broadcast(M[:], m1[:])
                if DEBUG and i == 0:
                    nc.sync.dma_start(out=dbg_R[0:1, :], in_=y[:])
                    nc.sync.dma_start(out=dbg_R[1:2, :], in_=m1[:])

                # Q projection on raw x; consumer is a plain psum->sbuf
                # copy so the proj-psum ring never waits on the LN chain.
                for half in range(2):
                    for nl in range(4):
                        n = half * 4 + nl
                        qp = proj_ps.tile([128, 512], F32, tag="proj")
                        for k in range(DT):
                            nc.tensor.matmul(
                                qp[:],
                                wq_sb[:, k, ts(half, 512)][:, ts(nl, 128)],
                                x[:, k, :],
                                start=(k == 0), stop=(k == DT - 1),
                            )
                        nc.vector.tensor_copy(qraw_sb[:, n, :], qp[:])
                # deferred per-token fixup: q = R*(qraw + c) + M*s (+ bq)
                for n in range(DT):
                    tq = tqp.tile([128, TQ], F32, tag="tq")
                    nc.vector.scalar_tensor_tensor(
                        tq[:], qraw_sb[:, n, :], bcs[:, 8 + n : 9 + n], R[:],
                        op0=OP.add, op1=OP.mult,
                    )
                    nc.vector.scalar_tensor_tensor(
                        qt_sb[:, n, :], M[:], bcs[:, 16 + n : 17 + n],
                        tq[:], op0=OP.mult, op1=OP.add,
                    )
                    if not SKIP_BQ:
                        nc.vector.tensor_scalar_add(
                            qt_sb[:, n, :], qt_sb[:, n, :],
                            bcs[:, 24 + n : 25 + n],
                        )

            # ---- attention for layer i, pulling fill work between groups ----
            def attention(i, fill):
                kt_sb = ktb[i % 2]
                vaug = vb[i % 2]
                last = i == L - 1
                x_next = None if last else xb[(i + 1) % 2]
                sq_tiles = []
                for hp in range(DT):
                    if last and hp == DT // 2:
                        drain(fill)
                    hA, hB = 2 * hp, 2 * hp + 1
                    avA = avA_ps.tile([HD + 1, TQ], F32, tag="avA")
                    avB = avB_ps.tile([HD + 1, TQ], F32, tag="avB")
                    pts = []

                    def av_one(kk):
                        pt = pts[kk]
                        nc.tensor.matmul(
                            avA[:], vaug[:, kk, hA, :], pt[:, 0, :],
                            start=(kk == 0), stop=(kk == KT - 1),
                        )
                        nc.tensor.matmul(
                            avB[:], vaug[:, kk, hB, :], pt[:, 1, :],
                            start=(kk == 0), stop=(kk == KT - 1),
                        )

                    for ktp in range(4):
                        scs = []
                        for sub in range(2):
                            kk = 2 * ktp + sub
                            sc = sc_ps.tile([128, 2, TQ], F32, tag="sc")
                            nc.tensor.matmul(
                                sc[:, 0, :],
                                kt_sb[0:64, hp, ts(kk, 128)],
                                qt_sb[0:64, hp, :],
                                start=True, stop=True,
                            )
                            nc.tensor.matmul(
                                sc[:, 1, :],
                                kt_sb[64:128, hp, ts(kk, 128)],
                                qt_sb[64:128, hp, :],
                                start=True, stop=True,
                            )
                            scs.append(sc)
                        for sub in range(2):
                            pt = ptp.tile([128, 2, TQ], BF16, tag="pt")
                            nc.scalar.activation(pt[:], scs[sub][:], AF.Exp)
                            pts.append(pt)
                        if ktp > 0:
                            av_one(2 * ktp - 2)
                            av_one(2 * ktp - 1)
                        pull(fill, 5)
                    av_one(KT - 2)
                    av_one(KT - 1)

                    # normalize: x[d,t] = av[d,t] / den[t]
                    st = stgp.tile([128, TQ], F32, tag="stage", name="st") if last else None
                    for av, o in ((avA, 0), (avB, 64)):
                        r0 = recipp.tile([1, TQ], F32, tag="r0")
                        nc.vector.tensor_copy(r0[:], av[HD : HD + 1, :])
                        nc.vector.reciprocal_approx_fast(r0[:], r0[:])
                        bc = bcp.tile([64, TQ], F32, tag="bc")
                        nc.gpsimd.partition_broadcast(bc[:], r0[:])
                        if not last:
                            nc.vector.tensor_tensor(
                                x_next[o : o + 64, hp, :], av[0:HD, :], bc[:],
                                op=OP.mult,
                            )
                        else:
                            nc.vector.tensor_tensor(
                                st[o : o + 64, :], av[0:HD, :], bc[:],
                                op=OP.mult,
                            )
                            nc.vector.tensor_scalar_add(
                                st[o : o + 64, :], st[o : o + 64, :],
                                bvf_sb[o : o + 64, hp : hp + 1],
                            )
                    if last:
                        nc.sync.dma_start(out=outr[:, hp, :], in_=st[:])
                    if not last:
                        sqt = sqp.tile([128, TQ], BF16, tag="sq")
                        if SKIP_BVP:
                            nc.vector.tensor_mul(
                                sqt[:], x_next[:, hp, :], x_next[:, hp, :]
                            )
                        else:
                            bcn = bc_sb[(i + 1) % 2]
                            tb = tbp.tile([128, TQ], BF16, tag="tb")
                            nc.vector.tensor_scalar_add(
                                tb[:], x_next[:, hp, :],
                                bcn[:, 34 + hp : 35 + hp],
                            )
                            nc.vector.tensor_mul(sqt[:], tb[:], tb[:])
                        sq_tiles.append(sqt)
                return sq_tiles

            def dbg_dump(dst, src_tile):
                nc.sync.dma_start(out=dst, in_=src_tile[:])

            # ================= schedule =================
            # prologue: layer 0 projections + LN + Q
            dma_consts(0)
            dma_w(wv_sb, wvd, 0)
            dma_w(wq_sb, wqd, 0)
            drain([kv_gen(0)])
            sq0 = []
            for k in range(DT):
                sqt = sqp.tile([128, TQ], BF16, tag="sq")
                if SKIP_BVP:
                    nc.vector.tensor_mul(
                        sqt[:], xb[0][:, k, :], xb[0][:, k, :]
                    )
                else:
                    tb = tbp.tile([128, TQ], BF16, tag="tb")
                    nc.vector.tensor_scalar_add(
                        tb[:], xb[0][:, k, :], bc_sb[0][:, 34 + k : 35 + k]
                    )
                    nc.vector.tensor_mul(sqt[:], tb[:], tb[:])
                sq0.append(sqt)
            ln_q(0, sq0)

            if DEBUG:
                dbg_dump(dbg_kt, ktb[0])
                dbg_dump(dbg_v, vb[0])
                dbg_dump(dbg_qt, qt_sb)
            fill = []
            for i in range(L):
                if i < L - 1:
                    dma_consts(i + 1)
                    dma_w(wk_sb, wkd, i + 1)
                    dma_w(wv_sb, wvd, i + 1)
                    dma_w(wq_sb, wqd, i + 1)
                    fill = [kv_gen(i + 1, (0,)), kv_gen(i + 1, (1,))]
                sq_tiles = attention(i, fill)
                if DEBUG and i == 0:
                    dbg_dump(dbg_x1, xb[1])
                if i < L - 1:
                    if i == L - 2:
                        # leave half-1 of layer-3 K/V as fill work for
                        # attention(3) itself (its hp 0-3 don't need it)
                        if len(fill) == 2:
                            drain(fill[:1])
                            fill = fill[1:]
                        else:
                            pass
                    else:
                        drain(fill)
                        fill = []
                    ln_q(i + 1, sq_tiles)

    nc.compile()
    _PROGRAM[spec] = nc
    return nc


def _stage_inputs(input_ids, encoder_state, cross_attn_mask, emb,
                  ln_g, ln_b, wq, bq, wk, bk, wv, bv):
    input_ids = np.asarray(input_ids)
    emb = np.asarray(emb, dtype=np.float32)
    encoder_state = np.asarray(encoder_state, dtype=np.float32)
    cross_attn_mask = np.asarray(cross_attn_mask, dtype=np.float32)
    ln_g = np.asarray(ln_g, dtype=np.float32)
    ln_b = np.asarray(ln_b, dtype=np.float32)
    wq = np.asarray(wq, dtype=np.float32)
    bq = np.asarray(bq, dtype=np.float32)
    wk = np.asarray(wk, dtype=np.float32)
    bk = np.asarray(bk, dtype=np.float32)
    wv = np.asarray(wv, dtype=np.float32)
    bv = np.asarray(bv, dtype=np.float32)

    scale = 1.0 / np.sqrt(HD)
    wq2 = ln_g[:, :, None] * wq * scale  # [L, D, D]
    bq2 = (np.einsum("ld,lde->le", ln_b, wq) + bq) * scale  # [L, D]
    bv_prev = np.concatenate([np.zeros((1, D), np.float32), bv[:-1]], axis=0)
    cq = np.einsum("lde,ld->le", wq2, bv_prev)  # [L, D]
    sq_ = wq2.sum(axis=1)  # [L, D]

    def pcol(a):  # [L, D] -> [L, 128, DT]
        return np.ascontiguousarray(a.reshape(-1, DT, 128).transpose(0, 2, 1))

    bcons = np.zeros((L, 128, 42), np.float32)
    bcons[:, :, 0:8] = pcol(bk)
    bcons[:, :, 8:16] = pcol(cq)
    bcons[:, :, 16:24] = pcol(sq_)
    bcons[:, :, 24:32] = pcol(bq2)
    bcons[:, 0, 32] = bv_prev.sum(axis=1)
    bcons[:, 0, 33] = D * EPS
    bcons[:, :, 34:42] = pcol(bv_prev)
    bf = ml_dtypes.bfloat16

    shared = {
        "wqd": wq2.astype(bf),
        "wkd": wk.astype(bf),
        "wvd": wv.astype(bf),
        "bcons": np.ascontiguousarray(bcons),
        "bvfd": np.ascontiguousarray(pcol(bv[L - 1 : L])[0]),
    }

    x0 = emb[input_ids]  # [NB, TQ, D]
    in_maps = []
    for n in range(NB):
        m = dict(shared)
        m["xt0"] = np.ascontiguousarray(x0[n].T).astype(bf)
        m["enct"] = np.ascontiguousarray(encoder_state[n].T).astype(bf)
        mk = cross_attn_mask[n, 0, 0]  # [TK]
        m["cvald"] = np.ascontiguousarray(
            np.exp(mk).reshape(KT, 128).T.astype(np.float32)
        )
        in_maps.append(m)
    return in_maps


def kernel(**inputs) -> np.ndarray:
    ln_b = np.asarray(inputs["ln_b"], np.float32)
    bq = np.asarray(inputs["bq"], np.float32)
    bv = np.asarray(inputs["bv"], np.float32)
    mask = np.asarray(inputs["cross_attn_mask"], np.float32)
    spec = (
        bool(not ln_b.any() and not bq.any()),
        bool(not bv[:-1].any()),
        bool(not mask.any()),
    )
    nc = build_program(spec)
    in_maps = _stage_inputs(**inputs)
    res = run_bass_kernel_spmd(nc, in_maps, list(range(NB)))
    out = np.stack([np.asarray(res.results[n]["outd"]).T for n in range(NB)])
    return np.ascontiguousarray(out, dtype=np.float32)


if __name__ == "__main__":
    build_program()
    print("program built ok")


# revision 22
# speedup vs baseline: 1.0503x; 1.0014x over previous
"""Trainium2 Bass kernel for nn_DecoderPreLN2 (4-layer cross-attention decoder).

Sharding: data-parallel over batch N=8 across 8 NeuronCores (1 element/core).
Per-core dataflow is fully "transposed" (D-major): activations live as x^T
[D=1024 partitions(8 tiles), T free] so every matmul keeps weights stationary
and no on-device transposes are needed.

v2 design (vs fp32r baseline):
  - All matmuls bf16 (measured same row-rate as fp32r but lighter LDWEIGHTS
    and half the DMA); PSUM accumulates f32.
  - The PE is kept *continuously busy* so the HAM clock gate stays at 8/8
    (2.4 GHz): K/V projections of layer i+1 are interleaved as "fill" matmuls
    into the ACT-gated gaps of attention(i).
  - exp(mask) is folded into the V ones-column (host computes exp(mask);
    the V-copy scales rows by it), so the score exp needs no bias operand
    and two k-tiles are merged per ACTIVATE (bf16 out).
  - LN affine + 1/sqrt(HD) folded into wq (host). LN statistics are computed
    on raw attention output via PE ones-matmuls into a proj-pool psum tile;
    the per-token LN scalars are applied as a *post*-fixup on the Q
    projection output (q = R*(qraw + c) + M*s + bq), so Q matmuls never wait
    on the LN chain.
  - V-bias bv is folded into the next layer's LN stats (softmax weights sum
    to 1); the last layer adds bv explicitly.
  - rstd via DVE-only fast-inverse-sqrt (bit trick + 2 Newton steps): no
    ACT table switching (ACT only ever loads the exp set).
  - Softmax denominator via the (scaled) ones-column of V; normalize =
    DVE reciprocal + GPSIMD partition_broadcast + DVE multiply.
  - Last layer streams the f32 output to HBM per head-pair tile.
"""

import os
import sys

sys.path.insert(0, "/opt/trn_rl_repo")

DEBUG = bool(int(os.environ.get("KERNEL_DEBUG", "0")))

import numpy as np
import ml_dtypes

import concourse.bass as bass
import concourse.tile as tile
from concourse import bacc, mybir
from concourse.bass import ts
from concourse.bass_utils import run_bass_kernel_spmd

L, D, H, HD = 4, 1024, 16, 64
TQ, TK, NB = 512, 1024, 8
DT = D // 128  # 8 d-tiles
KT = TK // 128  # 8 k-token tiles
EPS = 1e-5

F32 = mybir.dt.float32
BF16 = mybir.dt.bfloat16
I32 = mybir.dt.int32
AF = mybir.ActivationFunctionType
OP = mybir.AluOpType

RSQRT_MAGIC = 0x5F3759DF

_PROGRAM = {}


def build_program(spec=(True, True, True)):
    """spec = (skip_bq, skip_bvp, cval_ones) input-derived specializations."""
    global _PROGRAM
    if spec in _PROGRAM:
        return _PROGRAM[spec]
    SKIP_BQ, SKIP_BVP, CVAL_ONES = spec

    nc = bacc.Bacc(
        "TRN2", target_bir_lowering=False, debug=False,
        dynamic_dma_scratch_size=2048,
    )

    xt0 = nc.dram_tensor("xt0", [D, TQ], BF16, kind="ExternalInput").ap()
    enct = nc.dram_tensor("enct", [D, TK], BF16, kind="ExternalInput").ap()
    cvald = nc.dram_tensor("cvald", [128, KT], F32, kind="ExternalInput").ap()
    wqd = nc.dram_tensor("wqd", [L, D, D], BF16, kind="ExternalInput").ap()
    wkd = nc.dram_tensor("wkd", [L, D, D], BF16, kind="ExternalInput").ap()
    wvd = nc.dram_tensor("wvd", [L, D, D], BF16, kind="ExternalInput").ap()
    # per-layer f32 columns: 0-7 bk, 8-15 cq, 16-23 sq, 24-31 bq2,
    # [0,32]=sum(bvp), [0,33]=D*EPS, 34-41 bvp (pcol)
    bcons = nc.dram_tensor("bcons", [L, 128, 42], F32, kind="ExternalInput").ap()
    bvfd = nc.dram_tensor("bvfd", [128, DT], F32, kind="ExternalInput").ap()
    outd = nc.dram_tensor("outd", [D, TQ], F32, kind="ExternalOutput").ap()
    outr = outd.rearrange("(j p) t -> p j t", p=128)
    if DEBUG:
        dbg_kt = nc.dram_tensor("dbg_kt", [128, DT, TK], BF16, kind="ExternalOutput").ap()
        dbg_v = nc.dram_tensor("dbg_v", [128, KT, H, HD + 1], BF16, kind="ExternalOutput").ap()
        dbg_qt = nc.dram_tensor("dbg_qt", [128, DT, TQ], BF16, kind="ExternalOutput").ap()
        dbg_R = nc.dram_tensor("dbg_R", [2, TQ], F32, kind="ExternalOutput").ap()
        dbg_x1 = nc.dram_tensor("dbg_x1", [128, DT, TQ], BF16, kind="ExternalOutput").ap()
        dbg_pt = nc.dram_tensor("dbg_pt", [128, 2, 2, TQ], BF16, kind="ExternalOutput").ap()

    from contextlib import ExitStack

    with tile.TileContext(nc) as tc, ExitStack() as es:
            persist = es.enter_context(tc.tile_pool(name="persist", bufs=1))
            xp = es.enter_context(tc.tile_pool(name="xp", bufs=2))
            ktp = es.enter_context(tc.tile_pool(name="ktp", bufs=2))
            vp = es.enter_context(tc.tile_pool(name="vp", bufs=2))
            qtp = es.enter_context(tc.tile_pool(name="qtp", bufs=1))
            wkp = es.enter_context(tc.tile_pool(name="wkp", bufs=1))
            wvp = es.enter_context(tc.tile_pool(name="wvp", bufs=1))
            wqp = es.enter_context(tc.tile_pool(name="wqp", bufs=1))
            ptp = es.enter_context(tc.tile_pool(name="ptp", bufs=4))
            sqp = es.enter_context(tc.tile_pool(name="sqp", bufs=8))
            tbp = es.enter_context(tc.tile_pool(name="tbp", bufs=2))
            biasp = es.enter_context(tc.tile_pool(name="biasp", bufs=2))
            smalls = es.enter_context(tc.tile_pool(name="smalls", bufs=1))
            rmp = es.enter_context(tc.tile_pool(name="rmp", bufs=1))
            tqp = es.enter_context(tc.tile_pool(name="tqp", bufs=1))
            recipp = es.enter_context(tc.tile_pool(name="recipp", bufs=2))
            bcp = es.enter_context(tc.tile_pool(name="bcp", bufs=2))
            stgp = es.enter_context(tc.tile_pool(name="stgp", bufs=2))
            proj_ps = es.enter_context(
                tc.tile_pool(name="proj_ps", bufs=2, space="PSUM"))
            sc_ps = es.enter_context(
                tc.tile_pool(name="sc_ps", bufs=2, space="PSUM"))
            avA_ps = es.enter_context(
                tc.tile_pool(name="avA_ps", bufs=1, space="PSUM"))
            avB_ps = es.enter_context(
                tc.tile_pool(name="avB_ps", bufs=1, space="PSUM"))
            # ---- persistent tiles; startup DMAs chunked so the first
            # K-projection matmuls can begin after ~2 d-tiles arrive ----
            enc_sb = persist.tile([128, DT, TK], BF16, tag="enc")
            encr = enct.rearrange("(j p) t -> p j t", p=128)
            wk_sb = wkp.tile([128, DT, D], BF16, tag="wk")
            wkr0 = wkd[0].rearrange("(j p) o -> p j o", p=128)
            for k in range(DT):
                nc.sync.dma_start(out=enc_sb[:, k, :], in_=encr[:, k, :])
                nc.sync.dma_start(out=wk_sb[:, k, :], in_=wkr0[:, k, :])
            cval_sb = persist.tile([128, KT], F32, tag="cval")
            nc.sync.dma_start(out=cval_sb[:], in_=cvald[:])
            ones1 = persist.tile([128, 1], BF16, tag="ones1")
            nc.vector.memset(ones1[:], 1.0)
            bvf_sb = persist.tile([128, DT], F32, tag="bvf")
            nc.sync.dma_start(out=bvf_sb[:], in_=bvfd[:])

            xb = [xp.tile([128, DT, TQ], BF16, tag="x", name=f"x{j}")
                  for j in range(2)]
            nc.sync.dma_start(
                out=xb[0][:], in_=xt0.rearrange("(j p) t -> p j t", p=128)
            )
            ktb = [ktp.tile([128, DT, TK], BF16, tag="kt", name=f"kt{j}")
                   for j in range(2)]
            vb = [vp.tile([128, KT, H, HD + 1], BF16, tag="vaug", name=f"v{j}")
                  for j in range(2)]
            qt_sb = qtp.tile([128, DT, TQ], BF16, tag="qt")
            qraw_sb = qtp.tile([128, DT, TQ], BF16, tag="qraw")
            # scaled-ones column of v_aug (den weights): exp(mask) per token
            for j in range(2):
                for h in range(H):
                    nc.vector.tensor_copy(vb[j][:, :, h, HD : HD + 1], cval_sb[:])

            wv_sb = wvp.tile([128, DT, D], BF16, tag="wv")
            wq_sb = wqp.tile([128, DT, D], BF16, tag="wq")

            def dma_w(dst, wd, i):
                wr = wd[i].rearrange("(j p) o -> p j o", p=128)
                for k in range(DT):
                    nc.sync.dma_start(out=dst[:, k, :], in_=wr[:, k, :])

            bc_sb = [None, None]

            def dma_consts(i):
                b = biasp.tile([128, 42], F32, tag="bcons")
                nc.sync.dma_start(out=b[:], in_=bcons[i])
                bc_sb[i % 2] = b

            # ---- fill generator: K+V projections for layer i (one half:
            # heads [half*8, half*8+8) i.e. kt blocks n=half*4..half*4+3) ----
            def kv_gen(i, halves=(0, 1)):
                kt_dst = ktb[i % 2]
                v_dst = vb[i % 2]
                bcs = bc_sb[i % 2]
                for half in halves:
                    for nl in range(4):
                        n = half * 4 + nl
                        for c in range(2):
                            ps = proj_ps.tile([128, 512], F32, tag="proj")
                            for k in range(DT):
                                nc.tensor.matmul(
                                    ps[:],
                                    wk_sb[:, k, ts(half, 512)][:, ts(nl, 128)],
                                    enc_sb[:, k, ts(c, 512)],
                                    start=(k == 0), stop=(k == DT - 1),
                                )
                                yield 1
                            nc.vector.tensor_scalar_add(
                                kt_dst[:, n, ts(c, 512)], ps[:],
                                bcs[:, n : n + 1],
                            )
                    for m in range(KT):
                        ps = proj_ps.tile([128, 512], F32, tag="proj")
                        for k in range(DT):
                            nc.tensor.matmul(
                                ps[:], enc_sb[:, k, ts(m, 128)],
                                wv_sb[:, k, ts(half, 512)],
                                start=(k == 0), stop=(k == DT - 1),
                            )
                            yield 1
                        # scale token rows by exp(mask) while copying
                        if CVAL_ONES:
                            nc.vector.tensor_copy(
                                v_dst[:, m, ts(half, 8), 0:HD],
                                ps[:].rearrange("p (h e) -> p h e", h=8),
                            )
                        else:
                            nc.vector.tensor_scalar_mul(
                                v_dst[:, m, ts(half, 8), 0:HD],
                                ps[:].rearrange("p (h e) -> p h e", h=8),
                                cval_sb[:, m : m + 1],
                            )

            _DONE = object()

            def pull(gens, n):
                while n > 0 and gens:
                    if next(gens[0], _DONE) is _DONE:
                        gens.pop(0)
                    else:
                        n -= 1

            def drain(gens):
                while gens:
                    if next(gens[0], _DONE) is _DONE:
                        gens.pop(0)

            # ---- LN stats + smalls + Q projection + fixup for layer i ----
            def ln_q(i, sq_tiles):
                x = xb[i % 2]
                bcs = bc_sb[i % 2]
                lnt = proj_ps.tile([128, 512], F32, tag="proj")
                for k in range(DT):
                    nc.tensor.matmul(
                        lnt[0:1, :], ones1[:], x[:, k, :],
                        start=(k == 0), stop=(k == DT - 1),
                    )
                for k in range(DT):
                    nc.tensor.matmul(
                        lnt[32:33, :], ones1[:], sq_tiles[k][:],
                        start=(k == 0), stop=(k == DT - 1),
                    )
                # smalls: mu, var, rstd (fast inverse sqrt), M = -mu*rstd
                mu = smalls.tile([1, TQ], F32, tag="mu")
                nc.vector.tensor_scalar(
                    mu[:], lnt[0:1, :], bcs[0:1, 32:33], 1.0 / D,
                    op0=OP.add, op1=OP.mult,
                )
                e2 = smalls.tile([1, TQ], F32, tag="e2")
                nc.vector.tensor_scalar(
                    e2[:], lnt[32:33, :], bcs[0:1, 33:34], 1.0 / D,
                    op0=OP.add, op1=OP.mult,
                )
                msq = smalls.tile([1, TQ], F32, tag="msq")
                nc.vector.tensor_mul(msq[:], mu[:], mu[:])
                # e2 <- var = e2 - mu^2   (EPS folded into bcs[33] host-side)
                nc.vector.scalar_tensor_tensor(
                    e2[:], msq[:], -1.0, e2[:], op0=OP.mult, op1=OP.add
                )
                # fast inverse sqrt: y = bitcast(MAGIC - (v_int >> 1))
                y = smalls.tile([1, TQ], F32, tag="y")
                nc.vector.tensor_scalar(
                    y[:].bitcast(I32), e2[:].bitcast(I32), 1, -1,
                    op0=OP.logical_shift_right, op1=OP.bitwise_xor,
                )
                nc.vector.tensor_scalar_add(
                    y[:].bitcast(I32), y[:].bitcast(I32), RSQRT_MAGIC + 1
                )
                hv = smalls.tile([1, TQ], F32, tag="hv")
                nc.vector.tensor_scalar_mul(hv[:], e2[:], -0.5)
                u = smalls.tile([1, TQ], F32, tag="u")
                for _ in range(1):  # one Newton step (seed err 3.4% -> 0.17%)
                    nc.vector.tensor_mul(u[:], y[:], y[:])
                    nc.vector.tensor_mul(u[:], u[:], hv[:])
                    nc.vector.tensor_scalar_add(u[:], u[:], 1.5)
                    nc.vector.tensor_mul(y[:], y[:], u[:])
                m1 = smalls.tile([1, TQ], F32, tag="m1")
                nc.vector.scalar_tensor_tensor(
                    m1[:], mu[:], -1.0, y[:], op0=OP.mult, op1=OP.mult
                )
                R = rmp.tile([128, TQ], F32, tag="R")
                nc.gpsimd.partition_broadcast(R[:], y[:])
                M = rmp.tile([128, TQ], F32, tag="M")
                nc.gpsimd.partition_broadcast(M[:], m1[:])
                if DEBUG and i == 0:
                    nc.sync.dma_start(out=dbg_R[0:1, :], in_=y[:])
                    nc.sync.dma_start(out=dbg_R[1:2, :], in_=m1[:])

                # Q projection on raw x; consumer is a plain psum->sbuf
                # copy so the proj-psum ring never waits on the LN chain.
                for half in range(2):
                    for nl in range(4):
                        n = half * 4 + nl
                        qp = proj_ps.tile([128, 512], F32, tag="proj")
                        for k in range(DT):
                            nc.tensor.matmul(
                                qp[:],
                                wq_sb[:, k, ts(half, 512)][:, ts(nl, 128)],
                                x[:, k, :],
                                start=(k == 0), stop=(k == DT - 1),
                            )
                        nc.vector.tensor_copy(qraw_sb[:, n, :], qp[:])
                # deferred per-token fixup: q = R*(qraw + c) + M*s (+ bq)
                for n in range(DT):
                    tq = tqp.tile([128, TQ], F32, tag="tq")
                    nc.vector.scalar_tensor_tensor(
                        tq[:], qraw_sb[:, n, :], bcs[:, 8 + n : 9 + n], R[:],
                        op0=OP.add, op1=OP.mult,
                    )
                    nc.vector.scalar_tensor_tensor(
                        qt_sb[:, n, :], M[:], bcs[:, 16 + n : 17 + n],
                        tq[:], op0=OP.mult, op1=OP.add,
                    )
                    if not SKIP_BQ:
                        nc.vector.tensor_scalar_add(
                            qt_sb[:, n, :], qt_sb[:, n, :],
                            bcs[:, 24 + n : 25 + n],
                        )

            # ---- attention for layer i, pulling fill work between groups ----
            def attention(i, fill):
                kt_sb = ktb[i % 2]
                vaug = vb[i % 2]
                last = i == L - 1
                x_next = None if last else xb[(i + 1) % 2]
                sq_tiles = []
                for hp in range(DT):
                    if last and hp == DT // 2:
                        drain(fill)
                    hA, hB = 2 * hp, 2 * hp + 1
                    avA = avA_ps.tile([HD + 1, TQ], F32, tag="avA")
                    avB = avB_ps.tile([HD + 1, TQ], F32, tag="avB")
                    pts = []

                    def av_one(kk):
                        pt = pts[kk]
                        nc.tensor.matmul(
                            avA[:], vaug[:, kk, hA, :], pt[:, 0, :],
                            start=(kk == 0), stop=(kk == KT - 1),
                        )
                        nc.tensor.matmul(
                            avB[:], vaug[:, kk, hB, :], pt[:, 1, :],
                            start=(kk == 0), stop=(kk == KT - 1),
                        )

                    for ktp in range(4):
                        scs = []
                        for sub in range(2):
                            kk = 2 * ktp + sub
                            sc = sc_ps.tile([128, 2, TQ], F32, tag="sc")
                            nc.tensor.matmul(
                                sc[:, 0, :],
                                kt_sb[0:64, hp, ts(kk, 128)],
                                qt_sb[0:64, hp, :],
                                start=True, stop=True,
                            )
                            nc.tensor.matmul(
                                sc[:, 1, :],
                                kt_sb[64:128, hp, ts(kk, 128)],
                                qt_sb[64:128, hp, :],
                                start=True, stop=True,
                            )
                            scs.append(sc)
                        for sub in range(2):
                            pt = ptp.tile([128, 2, TQ], BF16, tag="pt")
                            nc.scalar.activation(pt[:], scs[sub][:], AF.Exp)
                            pts.append(pt)
                        if ktp > 0:
                            av_one(2 * ktp - 2)
                            av_one(2 * ktp - 1)
                        pull(fill, 5)
                    av_one(KT - 2)
                    av_one(KT - 1)

                    # normalize: x[d,t] = av[d,t] / den[t]
                    st = stgp.tile([128, TQ], F32, tag="stage", name="st") if last else None
                    for av, o in ((avA, 0), (avB, 64)):
                        r0 = recipp.tile([1, TQ], F32, tag="r0")
                        nc.vector.tensor_copy(r0[:], av[HD : HD + 1, :])
                        nc.vector.reciprocal_approx_fast(r0[:], r0[:])
                        bc = bcp.tile([64, TQ], F32, tag="bc")
                        nc.gpsimd.partition_broadcast(bc[:], r0[:])
                        if not last:
                            nc.vector.tensor_tensor(
                                x_next[o : o + 64, hp, :], av[0:HD, :], bc[:],
                                op=OP.mult,
                            )
                        else:
                            nc.vector.tensor_tensor(
                                st[o : o + 64, :], av[0:HD, :], bc[:],
                                op=OP.mult,
                            )
                            nc.vector.tensor_scalar_add(
                                st[o : o + 64, :], st[o : o + 64, :],
                                bvf_sb[o : o + 64, hp : hp + 1],
                            )
                    if last:
                        nc.sync.dma_start(out=outr[:, hp, :], in_=st[:])
                    if not last:
                        sqt = sqp.tile([128, TQ], BF16, tag="sq")
                        if SKIP_BVP:
                            nc.vector.tensor_mul(
                                sqt[:], x_next[:, hp, :], x_next[:, hp, :]
                            )
                        else:
                            bcn = bc_sb[(i + 1) % 2]
                            tb = tbp.tile([128, TQ], BF16, tag="tb")
                            nc.vector.tensor_scalar_add(
                                tb[:], x_next[:, hp, :],
                                bcn[:, 34 + hp : 35 + hp],
                            )
                            nc.vector.tensor_mul(sqt[:], tb[:], tb[:])
                        sq_tiles.append(sqt)
                return sq_tiles

            def dbg_dump(dst, src_tile):
                nc.sync.dma_start(out=dst, in_=src_tile[:])

            # ================= schedule =================
            # prologue: layer 0 projections + LN + Q
            dma_consts(0)
            dma_w(wv_sb, wvd, 0)
            dma_w(wq_sb, wqd, 0)
            drain([kv_gen(0)])
            sq0 = []
            for k in range(DT):
                sqt = sqp.tile([128, TQ], BF16, tag="sq")
                if SKIP_BVP:
                    nc.vector.tensor_mul(
                        sqt[:], xb[0][:, k, :], xb[0][:, k, :]
                    )
                else:
                    tb = tbp.tile([128, TQ], BF16, tag="tb")
                    nc.vector.tensor_scalar_add(
                        tb[:], xb[0][:, k, :], bc_sb[0][:, 34 + k : 35 + k]
                    )
                    nc.vector.tensor_mul(sqt[:], tb[:], tb[:])
                sq0.append(sqt)
            ln_q(0, sq0)

            if DEBUG:
                dbg_dump(dbg_kt, ktb[0])
                dbg_dump(dbg_v, vb[0])
                dbg_dump(dbg_qt, qt_sb)
            fill = []
            for i in range(L):
                if i < L - 1:
                    dma_consts(i + 1)
                    dma_w(wk_sb, wkd, i + 1)
                    dma_w(wv_sb, wvd, i + 1)
                    dma_w(wq_sb, wqd, i + 1)
                    fill = [kv_gen(i + 1, (0,)), kv_gen(i + 1, (1,))]
                sq_tiles = attention(i, fill)
                if DEBUG and i == 0:
                    dbg_dump(dbg_x1, xb[1])
                if i < L - 1:
                    if i == L - 2:
                        # leave half-1 of layer-3 K/V as fill work for
                        # attention(3) itself (its hp 0-3 don't need it)
                        if len(fill) == 2:
                            drain(fill[:1])
                            fill = fill[1:]
                        else:
                            pass
                    else:
                        drain(fill)
                        fill = []
                    ln_q(i + 1, sq_tiles)

    nc.compile()
    _PROGRAM[spec] = nc
    return nc


def _stage_inputs(input_ids, encoder_state, cross_attn_mask, emb,
                  ln_g, ln_b, wq, bq, wk, bk, wv, bv):
    input_ids = np.asarray(input_ids)
    emb = np.asarray(emb, dtype=np.float32)
    encoder_state = np.asarray(encoder_state, dtype=np.float32)
    cross_attn_mask = np.asarray(cross_attn_mask, dtype=np.float32)
    ln_g = np.asarray(ln_g, dtype=np.float32)
    ln_b = np.asarray(ln_b, dtype=np.float32)
    wq = np.asarray(wq, dtype=np.float32)
    bq = np.asarray(bq, dtype=np.float32)
    wk = np.asarray(wk, dtype=np.float32)
    bk = np.asarray(bk, dtype=np.float32)
    wv = np.asarray(wv, dtype=np.float32)
    bv = np.asarray(bv, dtype=np.float32)

    scale = 1.0 / np.sqrt(HD)
    wq2 = ln_g[:, :, None] * wq * scale  # [L, D, D]
    bq2 = (np.einsum("ld,lde->le", ln_b, wq) + bq) * scale  # [L, D]
    bv_prev = np.concatenate([np.zeros((1, D), np.float32), bv[:-1]], axis=0)
    cq = np.einsum("lde,ld->le", wq2, bv_prev)  # [L, D]
    sq_ = wq2.sum(axis=1)  # [L, D]

    def pcol(a):  # [L, D] -> [L, 128, DT]
        return np.ascontiguousarray(a.reshape(-1, DT, 128).transpose(0, 2, 1))

    bcons = np.zeros((L, 128, 42), np.float32)
    bcons[:, :, 0:8] = pcol(bk)
    bcons[:, :, 8:16] = pcol(cq)
    bcons[:, :, 16:24] = pcol(sq_)
    bcons[:, :, 24:32] = pcol(bq2)
    bcons[:, 0, 32] = bv_prev.sum(axis=1)
    bcons[:, 0, 33] = D * EPS
    bcons[:, :, 34:42] = pcol(bv_prev)
    bf = ml_dtypes.bfloat16

    shared = {
        "wqd": wq2.astype(bf),
        "wkd": wk.astype(bf),
        "wvd": wv.astype(bf),
        "bcons": np.ascontiguousarray(bcons),
        "bvfd": np.ascontiguousarray(pcol(bv[L - 1 : L])[0]),
    }

    x0 = emb[input_ids]  # [NB, TQ, D]
    in_maps = []
    for n in range(NB):
        m = dict(shared)
        m["xt0"] = np.ascontiguousarray(x0[n].T).astype(bf)
        m["enct"] = np.ascontiguousarray(encoder_state[n].T).astype(bf)
        mk = cross_attn_mask[n, 0, 0]  # [TK]
        m["cvald"] = np.ascontiguousarray(
            np.exp(mk).reshape(KT, 128).T.astype(np.float32)
        )
        in_maps.append(m)
    return in_maps


def kernel(**inputs) -> np.ndarray:
    ln_b = np.asarray(inputs["ln_b"], np.float32)
    bq = np.asarray(inputs["bq"], np.float32)
    bv = np.asarray(inputs["bv"], np.float32)
    mask = np.asarray(inputs["cross_attn_mask"], np.float32)
    spec = (
        bool(not ln_b.any() and not bq.any()),
        bool(not bv[:-1].any()),
        bool(not mask.any()),
    )
    nc = build_program(spec)
    in_maps = _stage_inputs(**inputs)
    res = run_bass_kernel_spmd(nc, in_maps, list(range(NB)))
    out = np.stack([np.asarray(res.results[n]["outd"]).T for n in range(NB)])
    return np.ascontiguousarray(out, dtype=np.float32)


if __name__ == "__main__":
    build_program()
    print("program built ok")
